# revision 32
# baseline (speedup 1.0000x reference)
"""Trainium2 Bass kernel for nn_AudioTransformer (neighborhood-attention transformer).

Strategy: sequence-parallel over 8 NeuronCores (64 tokens/core) with BATCHED
halo exchange: layers run in 4 segments of 2; per segment each core
redundantly computes a 3-tile working range (own tile +-1) for the first
layer (A) and just its own tile for the second (B), consuming a 5-tile K/V
range gathered once per segment. Only 3 AllGathers total (after layers 1, 3,
5); the full input is free. The own-token residual stream stays fp32 end to
end; only halo copies of h cross cores (bf16) and only affect attention
values.

Engine budget: matmuls on PE with bf16 operands wherever a reduction allows
(fp32 matmuls below 256 free-columns run at 1/4 rate); attention mask-adds on
the otherwise idle Pool engine; FF1/FF2 weight chunks stream on the ACT/DVE
DMA queues so the SP queue only carries QKV/proj weights and dynamic
gathers; masks are packed partition-major on the host so their DMA runs at
full element width.
"""

import numpy as np
import ml_dtypes

import concourse.bass as bass
import concourse.mybir as mybir
import concourse.tile as tile
from concourse.tile import add_dep_helper
from concourse import bacc
from concourse.bass_utils import run_bass_kernel_spmd


def _install_act_table_filter():
    """Make the act-table chooser resolve Ln/Exp/Identity/Copy only via the
    natural_log_exp_and_others set so each layer needs just 2 LUT swaps
    (to gelu_and_others and back) instead of 5."""
    import concourse.bacc as _bacc_mod
    if getattr(_bacc_mod, "_ant_act_filter", False):
        return
    _orig = _bacc_mod.get_activation_tables
    A = mybir.ActivationFunctionType
    movable = {A.Ln, A.Exp, A.Identity, A.Copy}

    def _filtered(arch):
        t = _orig(arch)
        out = {}
        for name, funcs in t.items():
            if name == "natural_log_exp_and_others":
                out[name] = set(funcs)
            else:
                out[name] = set(funcs) - movable
        return out

    _bacc_mod.get_activation_tables = _filtered
    _bacc_mod._ant_act_filter = True

BF = ml_dtypes.bfloat16
F32 = mybir.dt.float32
BF16 = mybir.dt.bfloat16

NC = 8          # cores
L = 512         # total tokens
LC = L // NC    # tokens per core/tile = 64
D = 512         # model dim
DT = D // 128   # 4 feature tiles
H = 8           # heads
DH = 64         # head dim
DFF = 2048      # ff dim
FT = DFF // 128  # 16 ff tiles
PATCH = 32
LAYERS = 8
SEGS = LAYERS // 2
K = 127         # neighborhood size
NEG = -60.0     # out-of-window logit bias (exp(-60+2) == 0 in fp32/bf16)

W5 = 5 * LC     # kv-range width (320)
P5 = 6 * LC     # padded kv width (384, 3 chunks of 128)
W3 = 3 * LC     # a-range width (192)
P3 = 4 * LC     # padded a-range width (256, 2 chunks of 128)

# wblob column offsets (per 128-row partition, bf16)
OFF_QKV = 0            # 4 fi-tiles x 1536
OFF_PROJ = 6144        # 4 fi-tiles x 512
OFF_FF1 = 8192         # fo-major [fo:16][fi:4][128]
OFF_FF2 = 16384        # fo-major [fo:4][g:16][128]
WCOLS = 24576

# pblob columns (f32)
PB_QKVB = 0    # 12
PB_PROJB = 12  # 4
PB_FF1B = 16   # 16
PB_FF2B = 32   # 4
PCOLS = 52

_BUILD_CACHE = {}


def _build():
    """Build + finalize the SPMD Bass graph (same graph on all 8 cores)."""
    _install_act_table_filter()
    nc = bacc.Bacc(None, target_bir_lowering=False)

    # ---- DRAM parameters (per-core inputs) ----
    xT = nc.dram_tensor("xT", [PATCH, LC], BF16, kind="ExternalInput")
    x5in = nc.dram_tensor("x5in", [PATCH, W5], BF16, kind="ExternalInput")
    w_in_T = nc.dram_tensor("w_in_T", [PATCH, D], BF16, kind="ExternalInput")
    inb = nc.dram_tensor("inb", [128, DT], F32, kind="ExternalInput")
    wblob = nc.dram_tensor("wblob", [LAYERS, 128, WCOLS], BF16, kind="ExternalInput")
    pblob = nc.dram_tensor("pblob", [LAYERS, 128, PCOLS], F32, kind="ExternalInput")
    vbias = nc.dram_tensor("vbias", [LAYERS, D], BF16, kind="ExternalInput")
    fbrow = nc.dram_tensor("fbrow", [LAYERS, DFF], BF16, kind="ExternalInput")
    maskA = nc.dram_tensor("maskA", [SEGS, 128, H, 3, W3], BF16,
                           kind="ExternalInput")
    maskB = nc.dram_tensor("maskB", [SEGS, 128, H, 2, LC], BF16,
                           kind="ExternalInput")
    w_out = nc.dram_tensor("w_out", [128, 128], BF16, kind="ExternalInput")
    outb = nc.dram_tensor("outb", [PATCH, 1], F32, kind="ExternalInput")
    yT = nc.dram_tensor("yT", [PATCH, LC], F32, kind="ExternalOutput")

    with tile.TileContext(nc) as tc:
        with (
            tc.tile_pool(name="singles", bufs=1) as singles,
            tc.tile_pool(name="wpool", bufs=2) as wpool,
            tc.tile_pool(name="mpool", bufs=1) as mpool,
            tc.tile_pool(name="bigpool", bufs=1) as bigpool,
            tc.tile_pool(name="actpool", bufs=2) as actpool,
            tc.tile_pool(name="tmppool", bufs=2) as tmppool,
            tc.tile_pool(name="statpool", bufs=1) as statpool,
            tc.tile_pool(name="agdram", bufs=2, space="DRAM") as agdram,
            # PSUM: 8 banks. pp:mm_out(3) + pp_ln(1) + ppv(1) + ppbc(1)
            #  + ppatt:ps_l(2) = 8
            tc.tile_pool(name="pp", bufs=3, space="PSUM") as pp,
            tc.tile_pool(name="pp_ln", bufs=1, space="PSUM") as pp_ln,
            tc.tile_pool(name="ppv", bufs=1, space="PSUM") as ppv,
            tc.tile_pool(name="ppatt", bufs=2, space="PSUM") as ppatt,
            tc.tile_pool(name="ppbc", bufs=1, space="PSUM") as ppbc,
        ):
            # persistent tiles
            hT_own = singles.tile([128, DT, LC], F32)   # own residual, f32
            hwork = singles.tile([128, DT, W5], F32)    # segment kv-range h
            ones_f = singles.tile([128, 1], F32)
            ones_b = singles.tile([128, 1], BF16)
            ones_row = singles.tile([1, 128], BF16)
            xin = singles.tile([PATCH, LC], BF16)
            x5s = singles.tile([PATCH, W5], BF16)
            win = singles.tile([PATCH, D], BF16)
            inb_s = singles.tile([128, DT], F32)
            wout_s = singles.tile([128, 128], BF16)
            outb_s = singles.tile([PATCH, 1], F32)
            pb_all = singles.tile([128, LAYERS, PCOLS], F32)

            nc.vector.memset(ones_f[:], 1.0)
            nc.vector.memset(ones_b[:], 1.0)
            nc.vector.memset(ones_row[:], 1.0)
            nc.sync.dma_start(xin[:], xT[:])
            nc.sync.dma_start(x5s[:], x5in[:])
            nc.sync.dma_start(win[:], w_in_T[:])
            nc.sync.dma_start(inb_s[:], inb[:])
            nc.sync.dma_start(wout_s[:], w_out[:])
            nc.sync.dma_start(outb_s[:], outb[:])
            nc.sync.dma_start(pb_all[:], pblob[:].rearrange("l p c -> p l c"))

            # per-core clip offsets as branch-free register arithmetic
            rank = nc.sync.partition_id()
            k0v = ((rank - 2) * ((rank >= 3) & (rank <= 5))
                   + 3 * (rank >= 6))                 # clip(c-2, 0, 3)
            a0v = ((rank - 1) * ((rank >= 1) & (rank <= 6))
                   + 5 * (rank >= 7))                 # clip(c-1, 0, 5)
            arv = a0v - k0v                           # a-range offset in kv range
            oiAv = rank - a0v                         # own tile within a-range
            oiv = rank - k0v                          # own tile within kv range

            def pbc(l, col):
                return pb_all[:, l, col:col + 1]

            def layernorm(srcT, sc0, srcbT, dstT, dc0, ncols, key):
                """normalize(srcT[:,:,sc0:+n]) -> dstT[:,:,dc0:+n] (bf16).
                srcbT: bf16 shadow tile of srcT (same columns) used for the
                PE reductions so they run at full bf16 rate."""
                src = srcT[:, :, sc0:sc0 + ncols]
                sqb = tmppool.tile([128, DT, ncols], BF16, tag=f"ln_sq{key}",
                                   bufs=1)
                nc.vector.tensor_mul(sqb[:], srcbT[:, :, sc0:sc0 + ncols],
                                     srcbT[:, :, sc0:sc0 + ncols])
                ps_s = pp_ln.tile([1, 512], F32, tag="sums", name="ps_s")
                for f in range(DT):
                    nc.tensor.matmul(ps_s[0:1, 0:ncols], ones_b[:],
                                     srcbT[:, f, sc0:sc0 + ncols],
                                     start=(f == 0), stop=(f == DT - 1))
                for f in range(DT):
                    nc.tensor.matmul(ps_s[0:1, 256:256 + ncols], ones_b[:],
                                     sqb[:, f, :],
                                     start=(f == 0), stop=(f == DT - 1))
                st = statpool.tile([1, 2 * ncols], F32, tag=f"ln_st{key}")
                nc.vector.tensor_scalar_mul(st[0:1, 0:ncols],
                                            ps_s[0:1, 0:ncols], 1.0 / D)
                m2 = statpool.tile([1, ncols], F32, tag=f"ln_m2{key}")
                nc.vector.tensor_mul(m2[:], st[0:1, 0:ncols], st[0:1, 0:ncols])
                nc.vector.tensor_scalar_add(m2[:], m2[:], -1e-5)
                var = statpool.tile([1, ncols], F32, tag=f"ln_var{key}")
                nc.vector.scalar_tensor_tensor(
                    var[:], ps_s[0:1, 256:256 + ncols], 1.0 / D, m2[:],
                    op0=mybir.AluOpType.mult, op1=mybir.AluOpType.subtract)
                # rstd = exp(-0.5*ln(var)) -- keeps ACT in the Ln/Exp func set
                sd = statpool.tile([1, ncols], F32, tag=f"ln_sd{key}")
                nc.scalar.activation(sd[:], var[:],
                                     mybir.ActivationFunctionType.Ln)
                nc.scalar.activation(st[0:1, ncols:2 * ncols], sd[:],
                                     mybir.ActivationFunctionType.Exp,
                                     scale=-0.5)
                stb = statpool.tile([1, 2 * ncols], BF16, tag=f"ln_stb{key}")
                nc.vector.tensor_copy(stb[:], st[:])
                # broadcast (mean, rstd) across partitions via K=1 matmul
                bc = ppbc.tile([128, 512], F32, tag="bcast", name="bc")
                nc.tensor.matmul(bc[:, 0:ncols], ones_row[:],
                                 stb[0:1, 0:ncols], start=True, stop=True)
                nc.tensor.matmul(bc[:, 256:256 + ncols], ones_row[:],
                                 stb[0:1, ncols:2 * ncols],
                                 start=True, stop=True)
                mean_w = bc[:, 0:ncols].unsqueeze(1).to_broadcast(
                    [128, DT, ncols])
                rstd_w = bc[:, 256:256 + ncols].unsqueeze(1).to_broadcast(
                    [128, DT, ncols])
                t0 = tmppool.tile([128, DT, ncols], F32, tag=f"ln_t0{key}",
                                  bufs=1)
                nc.vector.tensor_sub(t0[:], src, mean_w)
                nc.vector.tensor_mul(dstT[:, :, dc0:dc0 + ncols], t0[:],
                                     rstd_w)

            # ---- input projection ----
            for t in range(DT):
                ps = pp.tile([128, 2, W3], F32, tag="mm_out")
                nc.tensor.matmul(ps[:, 0, 0:LC], win[:, t * 128:(t + 1) * 128],
                                 xin[:], start=True, stop=True)
                nc.vector.tensor_scalar_add(hT_own[:, t, :], ps[:, 0, 0:LC],
                                            inb_s[:, t:t + 1])
            for t in range(DT):
                ps = pp.tile([128, 2, W3], F32, tag="mm_out")
                nc.tensor.matmul(ps[:, 0, 0:W3], win[:, t * 128:(t + 1) * 128],
                                 x5s[:, 0:W3], start=True, stop=True)
                nc.vector.tensor_scalar_add(hwork[:, t, 0:W3], ps[:, 0, 0:W3],
                                            inb_s[:, t:t + 1])
                ps2 = pp.tile([128, 2, W3], F32, tag="mm_out")
                nc.tensor.matmul(ps2[:, 0, 0:2 * LC],
                                 win[:, t * 128:(t + 1) * 128],
                                 x5s[:, W3:W5], start=True, stop=True)
                nc.vector.tensor_scalar_add(hwork[:, t, W3:W5],
                                            ps2[:, 0, 0:2 * LC],
                                            inb_s[:, t:t + 1])

            def load_layer(l):
                w_qkv = wpool.tile([128, 6144], BF16, tag="w_qkv", name="w_qkv")
                w_proj = wpool.tile([128, 2048], BF16, tag="w_proj",
                                    name="w_proj")
                vb = wpool.tile([1, D], BF16, tag="vb", name="vb")
                nc.sync.dma_start(w_qkv[:], wblob[l, :, OFF_QKV:OFF_PROJ])
                nc.sync.dma_start(w_proj[:], wblob[l, :, OFF_PROJ:OFF_FF1])
                nc.sync.dma_start(vb[:], vbias[l].unsqueeze(0))
                return w_qkv, w_proj, vb

            def load_ff1_chunk(l, ch):
                """4 fo-tiles of FF1 weights: [tt:4][fi:4][128] columns."""
                w = wpool.tile([128, 2048], BF16, tag="ff1c", name="ff1c",
                               bufs=3)
                nc.sync.dma_start(
                    w[:], wblob[l, :, OFF_FF1 + ch * 2048:
                                OFF_FF1 + (ch + 1) * 2048])
                return w

            def load_ff2_chunk(l, t):
                """One fo-tile of FF2 weights: [g:16][128] columns."""
                w = wpool.tile([128, 2048], BF16, tag="ff2c", name="ff2c",
                               bufs=3)
                nc.gpsimd.dma_start(
                    w[:], wblob[l, :, OFF_FF2 + t * 2048:
                                OFF_FF2 + (t + 1) * 2048])
                return w

            def attention_A(K5, V5, qT, mA, oT):
                """3 query tiles x 320-key window, chunked [128,128,64+pad]."""
                probs = []
                for h in range(H):
                    hh, g = h % 2, h // 2
                    ps12 = ppatt.tile([128, 2, W3], F32, tag="ps_l")
                    ps3p = ppv if h % 2 == 0 else ppbc
                    ps3 = ps3p.tile([128, D] if h % 2 == 0 else [128, 512],
                                    F32, tag="ps_v" if h % 2 == 0 else "bcast",
                                    name="ps3")
                    for kc in range(2):
                        nc.tensor.matmul(
                            ps12[:, kc, :],
                            K5[g][hh * DH:(hh + 1) * DH,
                                  kc * 128:(kc + 1) * 128],
                            qT[g][hh * DH:(hh + 1) * DH, :],
                            start=True, stop=True)
                    nc.tensor.matmul(
                        ps3[:, 0:W3],
                        K5[g][hh * DH:(hh + 1) * DH, 256:384],
                        qT[g][hh * DH:(hh + 1) * DH, :],
                        start=True, stop=True)
                    tmp_l = tmppool.tile([128, 3, W3], F32, tag="att_tmp", bufs=2)
                    nc.vector.tensor_add(tmp_l[:, 0:2, :], ps12[:],
                                         mA[:, h, 0:2, :])
                    nc.vector.tensor_add(tmp_l[:, 2, :], ps3[:, 0:W3],
                                         mA[:, h, 2, :])
                    probs_h = actpool.tile([128, 3, W3], BF16, tag=f"probs{h}",
                                           name="probs_h", bufs=1)
                    nc.scalar.activation(probs_h[:, 0:2, :], tmp_l[:, 0:2, :],
                                         mybir.ActivationFunctionType.Exp)
                    nc.scalar.activation(probs_h[:, 2, :], tmp_l[:, 2, :],
                                         mybir.ActivationFunctionType.Exp)
                    probs.append(probs_h)
                # denominators per query tile
                rs_bc = tmppool.tile([DH, H, 3, LC], F32, tag="rs_bc", bufs=1)
                for j in range(3):
                    ps_sum = pp_ln.tile([1, 512], F32, tag="sums",
                                        name="ps_sum")
                    for h in range(H):
                        for kc in range(3):
                            nc.tensor.matmul(
                                ps_sum[0:1, h * LC:(h + 1) * LC], ones_b[:],
                                probs[h][:, kc, j * LC:(j + 1) * LC],
                                start=(kc == 0), stop=(kc == 2))
                    rsum = statpool.tile([1, H * LC], F32, tag="rsum", bufs=2)
                    nc.vector.reciprocal(rsum[:], ps_sum[0:1, 0:H * LC])
                    rsb = statpool.tile([1, H * LC], BF16, tag="rsumb", bufs=2)
                    nc.vector.tensor_copy(rsb[:], rsum[:])
                    rs_ps = ppbc.tile([128, 512], F32, tag="bcast",
                                      name="rs_ps")
                    nc.tensor.matmul(rs_ps[0:DH, :], ones_row[0:1, 0:DH],
                                     rsb[:], start=True, stop=True)
                    nc.vector.tensor_copy(
                        rs_bc[:, :, j, :],
                        rs_ps[0:DH, :].rearrange("p (h q) -> p h q", q=LC))
                # AV + scale, N=192 per head
                for h in range(H):
                    hh, g = h % 2, h // 2
                    ps_o = pp.tile([128, 2, W3], F32, tag="mm_out", name="ps_o")
                    for kc in range(3):
                        nc.tensor.matmul(
                            ps_o[0:DH, 0, :],
                            V5[kc][:, h * DH:(h + 1) * DH],
                            probs[h][:, kc, :],
                            start=(kc == 0), stop=(kc == 2))
                    nc.vector.tensor_mul(
                        oT[g][hh * DH:(hh + 1) * DH, :], ps_o[0:DH, 0, :],
                        rs_bc[:, h].rearrange("p j q -> p (j q)"))

            def attention_B(K3, V3, qT, mB, oT):
                """1 query tile x 192-key window, chunks [128, 64+pad]."""
                probs = []
                for h in range(H):
                    hh, g = h % 2, h // 2
                    ps12 = ppatt.tile([128, 2, W3], F32, tag="ps_l")
                    for kc in range(2):
                        nc.tensor.matmul(
                            ps12[:, kc, 0:LC],
                            K3[g][hh * DH:(hh + 1) * DH,
                                  kc * 128:(kc + 1) * 128],
                            qT[g][hh * DH:(hh + 1) * DH, :],
                            start=True, stop=True)
                    tmp_l = tmppool.tile([128, 3, W3], F32, tag="att_tmp", bufs=2)
                    nc.vector.tensor_add(tmp_l[:, 0:2, 0:LC],
                                         ps12[:, :, 0:LC], mB[:, h, :, :])
                    probs_h = actpool.tile([128, 3, W3], BF16, tag=f"probs{h}",
                                           name="probs_h", bufs=1)
                    nc.scalar.activation(probs_h[:, 0:2, 0:LC],
                                         tmp_l[:, 0:2, 0:LC],
                                         mybir.ActivationFunctionType.Exp)
                    probs.append(probs_h)
                ps_sum = pp_ln.tile([1, 512], F32, tag="sums", name="ps_sum")
                for h in range(H):
                    for kc in range(2):
                        nc.tensor.matmul(
                            ps_sum[0:1, h * LC:(h + 1) * LC], ones_b[:],
                            probs[h][:, kc, 0:LC],
                            start=(kc == 0), stop=(kc == 1))
                rsum = statpool.tile([1, H * LC], F32, tag="rsum", bufs=2)
                nc.vector.reciprocal(rsum[:], ps_sum[0:1, 0:H * LC])
                rsb = statpool.tile([1, H * LC], BF16, tag="rsumb", bufs=2)
                nc.vector.tensor_copy(rsb[:], rsum[:])
                rs_ps = ppbc.tile([128, 512], F32, tag="bcast", name="rs_ps")
                nc.tensor.matmul(rs_ps[0:DH, :], ones_row[0:1, 0:DH],
                                 rsb[:], start=True, stop=True)
                rs_bc = tmppool.tile([DH, H, 3, LC], F32, tag="rs_bc", bufs=1)
                nc.vector.tensor_copy(
                    rs_bc[:, :, 0, :],
                    rs_ps[0:DH, :].rearrange("p (h q) -> p h q", q=LC))
                for h in range(H):
                    hh, g = h % 2, h // 2
                    ps_o = pp.tile([128, 2, W3], F32, tag="mm_out", name="ps_o")
                    for kc in range(2):
                        nc.tensor.matmul(
                            ps_o[0:DH, 0, 0:LC],
                            V3[kc][:, h * DH:(h + 1) * DH],
                            probs[h][:, kc, 0:LC],
                            start=(kc == 0), stop=(kc == 1))
                    nc.vector.tensor_mul(
                        oT[g][hh * DH:(hh + 1) * DH, :], ps_o[0:DH, 0, 0:LC],
                        rs_bc[:, h, 0, :])

            cur = load_layer(0)
            nxt = load_layer(1)
            hob = None
            for s in range(SEGS):
                lA, lB = 2 * s, 2 * s + 1
                w_qkv, w_proj, vb = cur
                mA = mpool.tile([128, H, 3, W3], BF16, tag="mA", name="mA")
                nc.sync.dma_start(mA[:], maskA[s])
                mB = mpool.tile([128, H, 2, LC], BF16, tag="mB", name="mB")
                nc.sync.dma_start(mB[:], maskB[s])

                hwb = bigpool.tile([128, DT, W5], BF16, tag="hwb")
                if s > 0:
                    # gather h_{lA-1} for the 5-tile kv range from ag_out
                    ag_out = ag_out_prev
                    for g in range(DT):
                        nc.sync.dma_start(
                            hwb[:, g, :].rearrange("p (r t) -> p r t", t=LC),
                            ag_out[bass.ds(k0v, 5),
                                   g * 128 * LC:(g + 1) * 128 * LC]
                            .rearrange("r (p t) -> p r t", t=LC))
                    nc.vector.tensor_copy(hwork[:], hwb[:])
                    # own tile stays f32-exact (and bf16 shadow from hob)
                    nc.sync.dma_start(
                        hwork[:, :, bass.ds(oiv * LC, LC)], hT_own[:])
                    nc.sync.dma_start(
                        hwb[:, :, bass.ds(oiv * LC, LC)], hob[:])
                else:
                    nc.vector.tensor_copy(hwb[:], hwork[:])

                # ---- layer A (3-tile working range, 5-tile kv range) ----
                x5 = bigpool.tile([128, DT, P5], BF16, tag="x5")
                nc.vector.memset(x5[:, :, W5:P5], 0.0)
                layernorm(hwork, 0, hwb, x5, 0, W3, "a")
                layernorm(hwork, W3, hwb, x5, W3, 2 * LC, "b")

                # hA = h values of the a-range (f32), own tile exact
                hA = bigpool.tile([128, DT, W3], F32, tag="hA")
                nc.sync.dma_start(hA[:], hwork[:, :, bass.ds(arv * LC, W3)])
                # xq = x~ of the a-range
                xq = actpool.tile([128, DT, W3], BF16, tag="xq")
                nc.sync.dma_start(xq[:], x5[:, :, bass.ds(arv * LC, W3)])

                # K for 5(+1 pad) tiles, feature-major per head-pair
                K5 = []
                for g in range(DT):
                    ps = ppv.tile([128, D], F32, tag="ps_v", name="ps_k5")
                    for h3 in range(3):
                        for f in range(DT):
                            nc.tensor.matmul(
                                ps[:, h3 * 128:(h3 + 1) * 128],
                                w_qkv[:, f * 1536 + 512 + g * 128:
                                      f * 1536 + 512 + (g + 1) * 128],
                                x5[:, f, h3 * 128:(h3 + 1) * 128],
                                start=(f == 0), stop=(f == DT - 1))
                    K5_g = actpool.tile([128, P5], BF16, tag=f"K5{g}",
                                        name="K5_g")
                    nc.vector.tensor_scalar_add(
                        K5_g[:], ps[:, 0:P5],
                        pbc(lA, PB_QKVB + DT + g))
                    K5.append(K5_g)
                # V for 3 chunks of 128 tokens, token-major
                V5 = []
                for kc in range(3):
                    ps_v = ppv.tile([128, D], F32, tag="ps_v")
                    for f in range(DT):
                        nc.tensor.matmul(
                            ps_v[:], x5[:, f, kc * 128:(kc + 1) * 128],
                            w_qkv[:, f * 1536 + 1024:f * 1536 + 1536],
                            start=(f == 0), stop=False)
                    nc.tensor.matmul(ps_v[:], ones_row[:], vb[:],
                                     start=False, stop=True)
                    V5_kc = actpool.tile([128, D], BF16, tag=f"V5{kc}",
                                         name="V5_kc")
                    nc.vector.tensor_copy(V5_kc[:], ps_v[:])
                    V5.append(V5_kc)

                # q for the 3 a-tiles
                qT = []
                for g in range(DT):
                    ps = pp.tile([128, 2, W3], F32, tag="mm_out")
                    for f in range(DT):
                        nc.tensor.matmul(
                            ps[:, 0, :],
                            w_qkv[:, f * 1536 + g * 128:f * 1536 + (g + 1) * 128],
                            xq[:, f, :], start=(f == 0), stop=(f == DT - 1))
                    qT_g = actpool.tile([128, W3], BF16, tag=f"qT{g}",
                                        name="qT_g")
                    nc.vector.tensor_scalar_add(
                        qT_g[:], ps[:, 0, :], pbc(lA, PB_QKVB + g))
                    qT.append(qT_g)

                oT = [actpool.tile([128, W3], BF16, tag=f"oT{g}", name="oT_g")
                      for g in range(DT)]
                attention_A(K5, V5, qT, mA, oT)

                # proj + residual into hA
                for t in range(DT):
                    ps = pp.tile([128, 2, W3], F32, tag="mm_out")
                    for f in range(DT):
                        nc.tensor.matmul(
                            ps[:, 0, :],
                            w_proj[:, f * 512 + t * 128:f * 512 + (t + 1) * 128],
                            oT[f][:], start=(f == 0), stop=(f == DT - 1))
                    nc.vector.scalar_tensor_tensor(
                        hA[:, t, :], ps[:, 0, :], pbc(lA, PB_PROJB + t),
                        hA[:, t, :], op0=mybir.AluOpType.add,
                        op1=mybir.AluOpType.add)

                # LN2 + FFN on the 3 a-tiles
                hAb = bigpool.tile([128, DT, W3], BF16, tag="hAb")
                nc.gpsimd.tensor_copy(hAb[:], hA[:])
                zA = bigpool.tile([128, DT, W3], BF16, tag="zA")
                layernorm(hA, 0, hAb, zA, 0, W3, "a")
                z1 = bigpool.tile([128, FT, W3], BF16, tag="z1")
                for ch in range(4):
                    wch = load_ff1_chunk(lA, ch)
                    for tl in range(4):
                        t = ch * 4 + tl
                        ps = pp.tile([128, 2, W3], F32, tag="mm_out",
                                     name="ps_ff1")
                        for f in range(DT):
                            nc.tensor.matmul(
                                ps[:, 0, :],
                                wch[:, tl * 512 + f * 128:
                                    tl * 512 + (f + 1) * 128],
                                zA[:, f, :], start=(f == 0),
                                stop=(f == DT - 1))
                        nc.scalar.activation(
                            z1[:, t, :], ps[:, 0, :],
                            mybir.ActivationFunctionType.Gelu,
                            bias=pb_all[:, lA, PB_FF1B + t:PB_FF1B + t + 1],
                            scale=1.0)
                for t in range(DT):
                    wch = load_ff2_chunk(lA, t)
                    ps = pp.tile([128, 2, W3], F32, tag="mm_out")
                    for g in range(FT):
                        nc.tensor.matmul(
                            ps[:, 0, :], wch[:, g * 128:(g + 1) * 128],
                            z1[:, g, :], start=(g == 0), stop=(g == FT - 1))
                    nc.vector.scalar_tensor_tensor(
                        hA[:, t, :], ps[:, 0, :], pbc(lA, PB_FF2B + t),
                        hA[:, t, :], op0=mybir.AluOpType.add,
                        op1=mybir.AluOpType.add)

                # ---- layer B (own tile; window = the 3 a-tiles) ----
                w_qkvB, w_projB, vbB = nxt
                if s + 1 < SEGS:
                    cur = load_layer(lA + 2)

                hAb2 = bigpool.tile([128, DT, W3], BF16, tag="hAb2")
                nc.gpsimd.tensor_copy(hAb2[:], hA[:])
                x3 = bigpool.tile([128, DT, P3], BF16, tag="x3")
                nc.vector.memset(x3[:, :, W3:P3], 0.0)
                layernorm(hA, 0, hAb2, x3, 0, W3, "a")

                own_hA = actpool.tile([128, DT, LC], F32, tag="own_hA")
                nc.sync.dma_start(own_hA[:], hA[:, :, bass.ds(oiAv * LC, LC)])
                own_xq = actpool.tile([128, DT, LC], BF16, tag="own_xq")
                nc.sync.dma_start(own_xq[:], x3[:, :, bass.ds(oiAv * LC, LC)])

                K3 = []
                for g in range(DT):
                    ps = ppv.tile([128, D], F32, tag="ps_v", name="ps_k3")
                    for h2 in range(2):
                        for f in range(DT):
                            nc.tensor.matmul(
                                ps[:, h2 * 128:(h2 + 1) * 128],
                                w_qkvB[:, f * 1536 + 512 + g * 128:
                                       f * 1536 + 512 + (g + 1) * 128],
                                x3[:, f, h2 * 128:(h2 + 1) * 128],
                                start=(f == 0), stop=(f == DT - 1))
                    K3_g = actpool.tile([128, P3], BF16, tag=f"K3{g}",
                                        name="K3_g")
                    nc.vector.tensor_scalar_add(
                        K3_g[:], ps[:, 0:P3], pbc(lB, PB_QKVB + DT + g))
                    K3.append(K3_g)
                V3 = []
                for kc in range(2):
                    ps_v = ppv.tile([128, D], F32, tag="ps_v")
                    for f in range(DT):
                        nc.tensor.matmul(
                            ps_v[:], x3[:, f, kc * 128:(kc + 1) * 128],
                            w_qkvB[:, f * 1536 + 1024:f * 1536 + 1536],
                            start=(f == 0), stop=False)
                    nc.tensor.matmul(ps_v[:], ones_row[:], vbB[:],
                                     start=False, stop=True)
                    V3_kc = actpool.tile([128, D], BF16, tag=f"V3{kc}",
                                         name="V3_kc")
                    nc.vector.tensor_copy(V3_kc[:], ps_v[:])
                    V3.append(V3_kc)

                qTB = []
                for g in range(DT):
                    ps = pp.tile([128, 2, W3], F32, tag="mm_out")
                    for f in range(DT):
                        nc.tensor.matmul(
                            ps[:, 0, 0:LC],
                            w_qkvB[:, f * 1536 + g * 128:f * 1536 + (g + 1) * 128],
                            own_xq[:, f, :], start=(f == 0), stop=(f == DT - 1))
                    qTB_g = actpool.tile([128, LC], BF16, tag=f"qTB{g}",
                                         name="qTB_g")
                    nc.vector.tensor_scalar_add(
                        qTB_g[:], ps[:, 0, 0:LC], pbc(lB, PB_QKVB + g))
                    qTB.append(qTB_g)

                oTB = [actpool.tile([128, LC], BF16, tag=f"oTB{g}",
                                    name="oTB_g") for g in range(DT)]
                attention_B(K3, V3, qTB, mB, oTB)

                # proj + residual: hT_own = own_hA + proj(oTB) + b
                for t in range(DT):
                    ps = pp.tile([128, 2, W3], F32, tag="mm_out")
                    for f in range(DT):
                        nc.tensor.matmul(
                            ps[:, 0, 0:LC],
                            w_projB[:, f * 512 + t * 128:f * 512 + (t + 1) * 128],
                            oTB[f][:], start=(f == 0), stop=(f == DT - 1))
                    nc.vector.scalar_tensor_tensor(
                        hT_own[:, t, :], ps[:, 0, 0:LC], pbc(lB, PB_PROJB + t),
                        own_hA[:, t, :], op0=mybir.AluOpType.add,
                        op1=mybir.AluOpType.add)

                hTb = actpool.tile([128, DT, LC], BF16, tag="hTb")
                nc.gpsimd.tensor_copy(hTb[:], hT_own[:])
                zB = bigpool.tile([128, DT, LC], BF16, tag="zB")
                layernorm(hT_own, 0, hTb, zB, 0, LC, "c")
                z1B = bigpool.tile([128, FT, LC], BF16, tag="z1B")
                for ch in range(4):
                    wch = load_ff1_chunk(lB, ch)
                    for tt in range(4):
                        t = ch * 4 + tt
                        ps = pp.tile([128, 2, W3], F32, tag="mm_out",
                                     name="ps_ff1b")
                        for f in range(DT):
                            nc.tensor.matmul(
                                ps[:, 0, 0:LC],
                                wch[:, tt * 512 + f * 128:
                                    tt * 512 + (f + 1) * 128],
                                zB[:, f, :], start=(f == 0),
                                stop=(f == DT - 1))
                        nc.scalar.activation(
                            z1B[:, t, :], ps[:, 0, 0:LC],
                            mybir.ActivationFunctionType.Gelu,
                            bias=pb_all[:, lB, PB_FF1B + t:PB_FF1B + t + 1],
                            scale=1.0)
                for t in range(DT):
                    wch = load_ff2_chunk(lB, t)
                    ps = pp.tile([128, 2, W3], F32, tag="mm_out")
                    for g in range(FT):
                        nc.tensor.matmul(
                            ps[:, 0, 0:LC], wch[:, g * 128:(g + 1) * 128],
                            z1B[:, g, :], start=(g == 0), stop=(g == FT - 1))
                    nc.vector.scalar_tensor_tensor(
                        hT_own[:, t, :], ps[:, 0, 0:LC], pbc(lB, PB_FF2B + t),
                        hT_own[:, t, :], op0=mybir.AluOpType.add,
                        op1=mybir.AluOpType.add)

                # ---- exchange h_{lB} (own tile) for the next segment ----
                if s + 1 < SEGS:
                    nxt = load_layer(lA + 3)
                    hob = actpool.tile([128, DT, LC], BF16, tag="hob")
                    nc.gpsimd.tensor_copy(hob[:], hT_own[:])
                    ag_in = agdram.tile([D * LC], BF16, tag="ag_in")
                    ag_out_prev = agdram.tile([NC, D * LC], BF16, tag="ag_out",
                                              addr_space="Shared")
                    nc.sync.dma_start(
                        ag_in[:].rearrange("(f p t) -> p f t", p=128, t=LC),
                        hob[:])
                    nc.gpsimd.collective_compute(
                        "AllGather", mybir.AluOpType.bypass,
                        ins=[ag_in[:].opt()], outs=[ag_out_prev[:].opt()],
                        replica_groups=[list(range(NC))])

            # ---- output projection: y.T = tanh(out_w @ hT_own + out_b) ----
            hb = actpool.tile([128, DT, LC], BF16, tag="hb")
            nc.vector.tensor_copy(hb[:], hT_own[:])
            ps_y = pp.tile([128, 2, W3], F32, tag="mm_out", name="ps_y")
            for f in range(DT):
                nc.tensor.matmul(ps_y[0:PATCH, 0, 0:LC],
                                 wout_s[:, f * PATCH:(f + 1) * PATCH],
                                 hb[:, f, :], start=(f == 0), stop=(f == DT - 1))
            y_sb = actpool.tile([PATCH, LC], F32, tag="y_sb")
            nc.scalar.activation(y_sb[:], ps_y[0:PATCH, 0, 0:LC],
                                 mybir.ActivationFunctionType.Tanh,
                                 bias=outb_s[:, 0:1], scale=1.0)
            nc.sync.dma_start(yT[:], y_sb[:])

    nc.finalize()
    return nc


def _prep_inputs(inputs):
    """Host-side: pack full fp32 inputs into per-core in_maps."""
    I = {k: np.asarray(v, np.float32) for k, v in inputs.items()}

    scale = np.float32(DH ** -0.5)
    qkv_w = I["qkv_w"].copy()          # [LAYERS, 3D, D]
    qkv_b = I["qkv_b"].copy()          # [LAYERS, 3D]
    ff1_w = I["ff1_w"].copy()          # [LAYERS, DFF, D]
    ff1_b = I["ff1_b"].copy()          # [LAYERS, DFF]
    for l in range(LAYERS):
        qkv_b[l] += qkv_w[l] @ I["ln1_b"][l]
        qkv_w[l] *= I["ln1_g"][l][None, :]
        ff1_b[l] += ff1_w[l] @ I["ln2_b"][l]
        ff1_w[l] *= I["ln2_g"][l][None, :]
    qkv_w[:, :D] *= scale
    qkv_b[:, :D] *= scale

    def part_major(m):
        X = m.shape[0] // 128
        return np.ascontiguousarray(
            m.reshape(X, 128, m.shape[1]).transpose(1, 0, 2).reshape(128, -1))

    wblob = np.empty((LAYERS, 128, WCOLS), BF)
    pblob = np.zeros((LAYERS, 128, PCOLS), np.float32)
    for l in range(LAYERS):
        qkvT = np.ascontiguousarray(qkv_w[l].T)          # [D, 3D]
        projT = np.ascontiguousarray(I["proj_w"][l].T)   # [D, D]
        ff1T = np.ascontiguousarray(ff1_w[l].T)          # [D, DFF]
        ff2T = np.ascontiguousarray(I["ff2_w"][l].T)     # [DFF, D]
        wblob[l, :, OFF_QKV:OFF_PROJ] = part_major(qkvT).astype(BF)
        wblob[l, :, OFF_PROJ:OFF_FF1] = part_major(projT).astype(BF)
        # FF regions fo-major so they stream in per-fo chunks
        wblob[l, :, OFF_FF1:OFF_FF2] = (
            part_major(ff1T).reshape(128, 4, 16, 128)
            .transpose(0, 2, 1, 3).reshape(128, 8192).astype(BF))
        wblob[l, :, OFF_FF2:WCOLS] = (
            part_major(ff2T).reshape(128, 16, 4, 128)
            .transpose(0, 2, 1, 3).reshape(128, 8192).astype(BF))
        pblob[l, :, PB_QKVB:PB_QKVB + 12] = qkv_b[l].reshape(12, 128).T
        pblob[l, :, PB_PROJB:PB_PROJB + 4] = I["proj_b"][l].reshape(4, 128).T
        pblob[l, :, PB_FF1B:PB_FF1B + 16] = ff1_b[l].reshape(16, 128).T
        pblob[l, :, PB_FF2B:PB_FF2B + 4] = I["ff2_b"][l].reshape(4, 128).T
    vbias = np.ascontiguousarray(qkv_b[:, 2 * D:3 * D]).astype(BF)
    fbrow = ff1_b.astype(BF)

    # attention bias+mask table over global (key, query) pairs
    i = np.arange(L)
    ni = np.clip(i - K // 2, 0, L - K)
    k_idx = np.arange(L)[:, None]
    in_win = (k_idx >= ni[None, :]) & (k_idx < (ni + K)[None, :])
    rel = np.clip(k_idx - i[None, :] + (K - 1), 0, 2 * K - 2)
    rpb = I["rpb"]                                       # [LAYERS, H, 2K-1]
    B_full = np.where(in_win[None, None], rpb[:, :, rel],
                      np.float32(NEG)).astype(np.float32)  # [LAYERS,H,L,L]

    w_in_T = np.ascontiguousarray(I["in_w"].T).astype(BF)
    inb = np.ascontiguousarray(I["in_b"].reshape(DT, 128).T)
    out_wT = np.ascontiguousarray(I["out_w"].T)
    w_out = part_major(out_wT).astype(BF)
    outb = np.ascontiguousarray(I["out_b"].reshape(PATCH, 1))

    x_tok = I["x"].reshape(L, PATCH)                     # [L, PATCH]

    in_maps = []
    for c in range(NC):
        k0 = min(max(c - 2, 0), 3)
        a0 = min(max(c - 1, 0), 5)
        xT_c = np.ascontiguousarray(x_tok[c * LC:(c + 1) * LC].T).astype(BF)
        x5_c = np.ascontiguousarray(
            x_tok[k0 * LC:(k0 + 5) * LC].T).astype(BF)   # [PATCH, 320]

        # masks packed partition-major: mA[s, p, h, kc, j*64+qi]
        mA = np.full((SEGS, 128, H, 3, W3), NEG, np.float32)
        mB = np.full((SEGS, 128, H, 2, LC), NEG, np.float32)
        for s in range(SEGS):
            blkA = B_full[2 * s, :, k0 * LC:k0 * LC + W5, :]   # [H, 320, L]
            for j in range(3):
                t = a0 + j
                sl = blkA[:, :, t * LC:(t + 1) * LC]           # [H, 320, LC]
                for kc in range(3):
                    lo, hi = kc * 128, min((kc + 1) * 128, W5)
                    # [H, rows, LC] -> [rows, H, LC]
                    mA[s, 0:hi - lo, :, kc, j * LC:(j + 1) * LC] = (
                        sl[:, lo:hi].transpose(1, 0, 2))
            blkB = B_full[2 * s + 1, :, a0 * LC:a0 * LC + W3,
                          c * LC:(c + 1) * LC]                 # [H, 192, LC]
            for kc in range(2):
                lo, hi = kc * 128, min((kc + 1) * 128, W3)
                mB[s, 0:hi - lo, :, kc, :] = blkB[:, lo:hi].transpose(1, 0, 2)

        in_maps.append({
            "xT": xT_c,
            "x5in": x5_c,
            "w_in_T": w_in_T,
            "inb": inb,
            "wblob": wblob,
            "pblob": pblob,
            "vbias": vbias,
            "fbrow": fbrow,
            "maskA": mA.astype(BF),
            "maskB": mB.astype(BF),
            "w_out": w_out,
            "outb": outb,
        })
    return in_maps


def kernel(**inputs):
    if "nc" not in _BUILD_CACHE:
        _BUILD_CACHE["nc"] = _build()
    nc = _BUILD_CACHE["nc"]
    in_maps = _prep_inputs(inputs)
    res = run_bass_kernel_spmd(nc, in_maps, core_ids=list(range(NC)))
    y = np.empty((1, 1, L * PATCH), np.float32)
    for c in range(NC):
        yT_c = res.results[c]["yT"]                      # [PATCH, LC]
        y[0, 0, c * LC * PATCH:(c + 1) * LC * PATCH] = yT_c.T.reshape(-1)
    return y


# revision 33
# speedup vs baseline: 1.0157x; 1.0157x over previous
"""Trainium2 Bass kernel for nn_AudioTransformer (neighborhood-attention transformer).

Strategy: sequence-parallel over 8 NeuronCores (64 tokens/core) with BATCHED
halo exchange: layers run in 4 segments of 2; per segment each core
redundantly computes a 3-tile working range (own tile +-1) for the first
layer (A) and just its own tile for the second (B), consuming a 5-tile K/V
range gathered once per segment. Only 3 AllGathers total (after layers 1, 3,
5); the full input is free. The own-token residual stream stays fp32 end to
end; only halo copies of h cross cores (bf16) and only affect attention
values.

Engine budget: matmuls on PE with bf16 operands wherever a reduction allows
(fp32 matmuls below 256 free-columns run at 1/4 rate); attention mask-adds on
the otherwise idle Pool engine; FF1/FF2 weight chunks stream on the ACT/DVE
DMA queues so the SP queue only carries QKV/proj weights and dynamic
gathers; masks are packed partition-major on the host so their DMA runs at
full element width.
"""

import numpy as np
import ml_dtypes

import concourse.bass as bass
import concourse.mybir as mybir
import concourse.tile as tile
from concourse.tile import add_dep_helper
from concourse import bacc
from concourse.bass_utils import run_bass_kernel_spmd


def _install_act_table_filter():
    """Make the act-table chooser resolve Ln/Exp/Identity/Copy only via the
    natural_log_exp_and_others set so each layer needs just 2 LUT swaps
    (to gelu_and_others and back) instead of 5."""
    import concourse.bacc as _bacc_mod
    if getattr(_bacc_mod, "_ant_act_filter", False):
        return
    _orig = _bacc_mod.get_activation_tables
    A = mybir.ActivationFunctionType
    movable = {A.Ln, A.Exp, A.Identity, A.Copy}

    def _filtered(arch):
        t = _orig(arch)
        out = {}
        for name, funcs in t.items():
            if name == "natural_log_exp_and_others":
                out[name] = set(funcs)
            else:
                out[name] = set(funcs) - movable
        return out

    _bacc_mod.get_activation_tables = _filtered
    _bacc_mod._ant_act_filter = True

BF = ml_dtypes.bfloat16
F32 = mybir.dt.float32
BF16 = mybir.dt.bfloat16

NC = 8          # cores
L = 512         # total tokens
LC = L // NC    # tokens per core/tile = 64
D = 512         # model dim
DT = D // 128   # 4 feature tiles
H = 8           # heads
DH = 64         # head dim
DFF = 2048      # ff dim
FT = DFF // 128  # 16 ff tiles
PATCH = 32
LAYERS = 8
SEGS = LAYERS // 2
K = 127         # neighborhood size
NEG = -60.0     # out-of-window logit bias (exp(-60+2) == 0 in fp32/bf16)

W5 = 5 * LC     # kv-range width (320)
P5 = 6 * LC     # padded kv width (384, 3 chunks of 128)
W3 = 3 * LC     # a-range width (192)
P3 = 4 * LC     # padded a-range width (256, 2 chunks of 128)

# wblob column offsets (per 128-row partition, bf16)
OFF_QKV = 0            # 4 fi-tiles x 1536
OFF_PROJ = 6144        # 4 fi-tiles x 512
OFF_FF1 = 8192         # fo-major [fo:16][fi:4][128]
OFF_FF2 = 16384        # fo-major [fo:4][g:16][128]
WCOLS = 24576

# pblob columns (f32)
PB_QKVB = 0    # 12
PB_PROJB = 12  # 4
PB_FF1B = 16   # 16
PB_FF2B = 32   # 4
PCOLS = 52

_BUILD_CACHE = {}


def _build():
    """Build + finalize the SPMD Bass graph (same graph on all 8 cores)."""
    _install_act_table_filter()
    nc = bacc.Bacc(None, target_bir_lowering=False)

    # ---- DRAM parameters (per-core inputs) ----
    xT = nc.dram_tensor("xT", [PATCH, LC], BF16, kind="ExternalInput")
    x5in = nc.dram_tensor("x5in", [PATCH, W5], BF16, kind="ExternalInput")
    w_in_T = nc.dram_tensor("w_in_T", [PATCH, D], BF16, kind="ExternalInput")
    inb = nc.dram_tensor("inb", [128, DT], F32, kind="ExternalInput")
    wblob = nc.dram_tensor("wblob", [LAYERS, 128, WCOLS], BF16, kind="ExternalInput")
    pblob = nc.dram_tensor("pblob", [LAYERS, 128, PCOLS], F32, kind="ExternalInput")
    vbias = nc.dram_tensor("vbias", [LAYERS, D], BF16, kind="ExternalInput")
    fbrow = nc.dram_tensor("fbrow", [LAYERS, DFF], BF16, kind="ExternalInput")
    maskA = nc.dram_tensor("maskA", [SEGS, 128, H, 3, W3], BF16,
                           kind="ExternalInput")
    maskB = nc.dram_tensor("maskB", [SEGS, 128, H, 2, LC], BF16,
                           kind="ExternalInput")
    w_out = nc.dram_tensor("w_out", [128, 128], BF16, kind="ExternalInput")
    outb = nc.dram_tensor("outb", [PATCH, 1], F32, kind="ExternalInput")
    yT = nc.dram_tensor("yT", [PATCH, LC], F32, kind="ExternalOutput")

    with tile.TileContext(nc) as tc:
        with (
            tc.tile_pool(name="singles", bufs=1) as singles,
            tc.tile_pool(name="wpool", bufs=2) as wpool,
            tc.tile_pool(name="mpool", bufs=1) as mpool,
            tc.tile_pool(name="bigpool", bufs=1) as bigpool,
            tc.tile_pool(name="actpool", bufs=2) as actpool,
            tc.tile_pool(name="tmppool", bufs=2) as tmppool,
            tc.tile_pool(name="statpool", bufs=1) as statpool,
            tc.tile_pool(name="agdram", bufs=2, space="DRAM") as agdram,
            # PSUM: 8 banks. pp:mm_out(3) + pp_ln(1) + ppv(1) + ppbc(1)
            #  + ppatt:ps_l(2) = 8
            tc.tile_pool(name="pp", bufs=3, space="PSUM") as pp,
            tc.tile_pool(name="pp_ln", bufs=1, space="PSUM") as pp_ln,
            tc.tile_pool(name="ppv", bufs=1, space="PSUM") as ppv,
            tc.tile_pool(name="ppatt", bufs=2, space="PSUM") as ppatt,
            tc.tile_pool(name="ppbc", bufs=1, space="PSUM") as ppbc,
        ):
            # persistent tiles
            hT_own = singles.tile([128, DT, LC], F32)   # own residual, f32
            hwork = singles.tile([128, DT, W5], F32)    # segment kv-range h
            ones_f = singles.tile([128, 1], F32)
            ones_b = singles.tile([128, 1], BF16)
            ones_row = singles.tile([1, 128], BF16)
            xin = singles.tile([PATCH, LC], BF16)
            x5s = singles.tile([PATCH, W5], BF16)
            win = singles.tile([PATCH, D], BF16)
            inb_s = singles.tile([128, DT], F32)
            wout_s = singles.tile([128, 128], BF16)
            outb_s = singles.tile([PATCH, 1], F32)
            pb_all = singles.tile([128, LAYERS, PCOLS], F32)

            nc.vector.memset(ones_f[:], 1.0)
            nc.vector.memset(ones_b[:], 1.0)
            nc.vector.memset(ones_row[:], 1.0)
            nc.sync.dma_start(xin[:], xT[:])
            nc.sync.dma_start(x5s[:], x5in[:])
            nc.sync.dma_start(win[:], w_in_T[:])
            nc.sync.dma_start(inb_s[:], inb[:])
            nc.sync.dma_start(wout_s[:], w_out[:])
            nc.sync.dma_start(outb_s[:], outb[:])
            nc.sync.dma_start(pb_all[:], pblob[:].rearrange("l p c -> p l c"))

            # per-core clip offsets as branch-free register arithmetic
            rank = nc.sync.partition_id()
            k0v = ((rank - 2) * ((rank >= 3) & (rank <= 5))
                   + 3 * (rank >= 6))                 # clip(c-2, 0, 3)
            a0v = ((rank - 1) * ((rank >= 1) & (rank <= 6))
                   + 5 * (rank >= 7))                 # clip(c-1, 0, 5)
            arv = a0v - k0v                           # a-range offset in kv range
            oiAv = rank - a0v                         # own tile within a-range
            oiv = rank - k0v                          # own tile within kv range

            def pbc(l, col):
                return pb_all[:, l, col:col + 1]

            def layernorm(srcT, sc0, srcbT, dstT, dc0, ncols, key):
                """normalize(srcT[:,:,sc0:+n]) -> dstT[:,:,dc0:+n] (bf16).
                srcbT: bf16 shadow tile of srcT (same columns) used for the
                PE reductions so they run at full bf16 rate."""
                src = srcT[:, :, sc0:sc0 + ncols]
                sqb = tmppool.tile([128, DT, ncols], BF16, tag=f"ln_sq{key}",
                                   bufs=1)
                nc.vector.tensor_mul(sqb[:], srcbT[:, :, sc0:sc0 + ncols],
                                     srcbT[:, :, sc0:sc0 + ncols])
                ps_s = pp_ln.tile([1, 512], F32, tag="sums", name="ps_s")
                for f in range(DT):
                    nc.tensor.matmul(ps_s[0:1, 0:ncols], ones_b[:],
                                     srcbT[:, f, sc0:sc0 + ncols],
                                     start=(f == 0), stop=(f == DT - 1))
                for f in range(DT):
                    nc.tensor.matmul(ps_s[0:1, 256:256 + ncols], ones_b[:],
                                     sqb[:, f, :],
                                     start=(f == 0), stop=(f == DT - 1))
                st = statpool.tile([1, 2 * ncols], F32, tag=f"ln_st{key}")
                nc.vector.tensor_scalar_mul(st[0:1, 0:ncols],
                                            ps_s[0:1, 0:ncols], 1.0 / D)
                m2 = statpool.tile([1, ncols], F32, tag=f"ln_m2{key}")
                nc.vector.tensor_mul(m2[:], st[0:1, 0:ncols], st[0:1, 0:ncols])
                nc.vector.tensor_scalar_add(m2[:], m2[:], -1e-5)
                var = statpool.tile([1, ncols], F32, tag=f"ln_var{key}")
                nc.vector.scalar_tensor_tensor(
                    var[:], ps_s[0:1, 256:256 + ncols], 1.0 / D, m2[:],
                    op0=mybir.AluOpType.mult, op1=mybir.AluOpType.subtract)
                # rstd = exp(-0.5*ln(var)) -- keeps ACT in the Ln/Exp func set
                sd = statpool.tile([1, ncols], F32, tag=f"ln_sd{key}")
                nc.scalar.activation(sd[:], var[:],
                                     mybir.ActivationFunctionType.Ln)
                nc.scalar.activation(st[0:1, ncols:2 * ncols], sd[:],
                                     mybir.ActivationFunctionType.Exp,
                                     scale=-0.5)
                stb = statpool.tile([1, 2 * ncols], BF16, tag=f"ln_stb{key}")
                nc.vector.tensor_copy(stb[:], st[:])
                # broadcast (mean, rstd) across partitions via K=1 matmul
                bc = ppbc.tile([128, 512], F32, tag="bcast", name="bc")
                nc.tensor.matmul(bc[:, 0:ncols], ones_row[:],
                                 stb[0:1, 0:ncols], start=True, stop=True)
                nc.tensor.matmul(bc[:, 256:256 + ncols], ones_row[:],
                                 stb[0:1, ncols:2 * ncols],
                                 start=True, stop=True)
                mean_w = bc[:, 0:ncols].unsqueeze(1).to_broadcast(
                    [128, DT, ncols])
                rstd_w = bc[:, 256:256 + ncols].unsqueeze(1).to_broadcast(
                    [128, DT, ncols])
                t0 = tmppool.tile([128, DT, ncols], F32, tag=f"ln_t0{key}",
                                  bufs=1)
                nc.vector.tensor_sub(t0[:], src, mean_w)
                nc.vector.tensor_mul(dstT[:, :, dc0:dc0 + ncols], t0[:],
                                     rstd_w)

            # ---- input projection ----
            for t in range(DT):
                ps = pp.tile([128, 2, W3], F32, tag="mm_out")
                nc.tensor.matmul(ps[:, 0, 0:LC], win[:, t * 128:(t + 1) * 128],
                                 xin[:], start=True, stop=True)
                nc.vector.tensor_scalar_add(hT_own[:, t, :], ps[:, 0, 0:LC],
                                            inb_s[:, t:t + 1])
            for t in range(DT):
                ps = pp.tile([128, 2, W3], F32, tag="mm_out")
                nc.tensor.matmul(ps[:, 0, 0:W3], win[:, t * 128:(t + 1) * 128],
                                 x5s[:, 0:W3], start=True, stop=True)
                nc.vector.tensor_scalar_add(hwork[:, t, 0:W3], ps[:, 0, 0:W3],
                                            inb_s[:, t:t + 1])
                ps2 = pp.tile([128, 2, W3], F32, tag="mm_out")
                nc.tensor.matmul(ps2[:, 0, 0:2 * LC],
                                 win[:, t * 128:(t + 1) * 128],
                                 x5s[:, W3:W5], start=True, stop=True)
                nc.vector.tensor_scalar_add(hwork[:, t, W3:W5],
                                            ps2[:, 0, 0:2 * LC],
                                            inb_s[:, t:t + 1])

            def load_layer(l):
                w_qkv = wpool.tile([128, 6144], BF16, tag="w_qkv", name="w_qkv")
                w_proj = wpool.tile([128, 2048], BF16, tag="w_proj",
                                    name="w_proj")
                vb = wpool.tile([1, D], BF16, tag="vb", name="vb")
                nc.sync.dma_start(w_qkv[:], wblob[l, :, OFF_QKV:OFF_PROJ])
                nc.sync.dma_start(w_proj[:], wblob[l, :, OFF_PROJ:OFF_FF1])
                nc.sync.dma_start(vb[:], vbias[l].unsqueeze(0))
                return w_qkv, w_proj, vb

            def load_ff1_chunk(l, ch):
                """4 fo-tiles of FF1 weights: [tt:4][fi:4][128] columns."""
                w = wpool.tile([128, 2048], BF16, tag="ff1c", name="ff1c",
                               bufs=3)
                nc.sync.dma_start(
                    w[:], wblob[l, :, OFF_FF1 + ch * 2048:
                                OFF_FF1 + (ch + 1) * 2048])
                return w

            def load_ff2_chunk(l, t):
                """One fo-tile of FF2 weights: [g:16][128] columns."""
                w = wpool.tile([128, 2048], BF16, tag="ff2c", name="ff2c",
                               bufs=3)
                nc.gpsimd.dma_start(
                    w[:], wblob[l, :, OFF_FF2 + t * 2048:
                                OFF_FF2 + (t + 1) * 2048])
                return w

            def attention_A(K5, V5, qT, mA, oT):
                """3 query tiles x 320-key window, chunked [128,128,64+pad]."""
                probs = []
                for h in range(H):
                    hh, g = h % 2, h // 2
                    ps12 = ppatt.tile([128, 2, W3], F32, tag="ps_l")
                    ps3p = ppv if h % 2 == 0 else ppbc
                    ps3 = ps3p.tile([128, D] if h % 2 == 0 else [128, 512],
                                    F32, tag="ps_v" if h % 2 == 0 else "bcast",
                                    name="ps3")
                    for kc in range(2):
                        nc.tensor.matmul(
                            ps12[:, kc, :],
                            K5[g][hh * DH:(hh + 1) * DH,
                                  kc * 128:(kc + 1) * 128],
                            qT[g][hh * DH:(hh + 1) * DH, :],
                            start=True, stop=True)
                    nc.tensor.matmul(
                        ps3[:, 0:W3],
                        K5[g][hh * DH:(hh + 1) * DH, 256:384],
                        qT[g][hh * DH:(hh + 1) * DH, :],
                        start=True, stop=True)
                    pe = tmppool.tile([128, 3, W3], BF16, tag="att_e",
                                      bufs=2)
                    nc.scalar.activation(pe[:, 0:2, :], ps12[:],
                                         mybir.ActivationFunctionType.Exp)
                    nc.scalar.activation(pe[:, 2, :], ps3[:, 0:W3],
                                         mybir.ActivationFunctionType.Exp)
                    probs_h = actpool.tile([128, 3, W3], BF16, tag=f"probs{h}",
                                           name="probs_h", bufs=1)
                    nc.gpsimd.tensor_mul(probs_h[:], pe[:], mA[:, h, :, :])
                    probs.append(probs_h)
                # denominators per query tile
                rs_bc = tmppool.tile([DH, H, 3, LC], F32, tag="rs_bc", bufs=1)
                for j in range(3):
                    ps_sum = pp_ln.tile([1, 512], F32, tag="sums",
                                        name="ps_sum")
                    for h in range(H):
                        for kc in range(3):
                            nc.tensor.matmul(
                                ps_sum[0:1, h * LC:(h + 1) * LC], ones_b[:],
                                probs[h][:, kc, j * LC:(j + 1) * LC],
                                start=(kc == 0), stop=(kc == 2))
                    rsum = statpool.tile([1, H * LC], F32, tag="rsum", bufs=2)
                    nc.vector.reciprocal(rsum[:], ps_sum[0:1, 0:H * LC])
                    rsb = statpool.tile([1, H * LC], BF16, tag="rsumb", bufs=2)
                    nc.vector.tensor_copy(rsb[:], rsum[:])
                    rs_ps = ppbc.tile([128, 512], F32, tag="bcast",
                                      name="rs_ps")
                    nc.tensor.matmul(rs_ps[0:DH, :], ones_row[0:1, 0:DH],
                                     rsb[:], start=True, stop=True)
                    nc.vector.tensor_copy(
                        rs_bc[:, :, j, :],
                        rs_ps[0:DH, :].rearrange("p (h q) -> p h q", q=LC))
                # AV + scale, N=192 per head
                for h in range(H):
                    hh, g = h % 2, h // 2
                    ps_o = pp.tile([128, 2, W3], F32, tag="mm_out", name="ps_o")
                    for kc in range(3):
                        nc.tensor.matmul(
                            ps_o[0:DH, 0, :],
                            V5[kc][:, h * DH:(h + 1) * DH],
                            probs[h][:, kc, :],
                            start=(kc == 0), stop=(kc == 2))
                    nc.vector.tensor_mul(
                        oT[g][hh * DH:(hh + 1) * DH, :], ps_o[0:DH, 0, :],
                        rs_bc[:, h].rearrange("p j q -> p (j q)"))

            def attention_B(K3, V3, qT, mB, oT):
                """1 query tile x 192-key window, chunks [128, 64+pad]."""
                probs = []
                for h in range(H):
                    hh, g = h % 2, h // 2
                    ps12 = ppatt.tile([128, 2, W3], F32, tag="ps_l")
                    for kc in range(2):
                        nc.tensor.matmul(
                            ps12[:, kc, 0:LC],
                            K3[g][hh * DH:(hh + 1) * DH,
                                  kc * 128:(kc + 1) * 128],
                            qT[g][hh * DH:(hh + 1) * DH, :],
                            start=True, stop=True)
                    pe = tmppool.tile([128, 3, W3], BF16, tag="att_e",
                                      bufs=2)
                    nc.scalar.activation(pe[:, 0:2, 0:LC], ps12[:, :, 0:LC],
                                         mybir.ActivationFunctionType.Exp)
                    probs_h = actpool.tile([128, 3, W3], BF16, tag=f"probs{h}",
                                           name="probs_h", bufs=1)
                    nc.gpsimd.tensor_mul(probs_h[:, 0:2, 0:LC],
                                         pe[:, 0:2, 0:LC], mB[:, h, :, :])
                    probs.append(probs_h)
                ps_sum = pp_ln.tile([1, 512], F32, tag="sums", name="ps_sum")
                for h in range(H):
                    for kc in range(2):
                        nc.tensor.matmul(
                            ps_sum[0:1, h * LC:(h + 1) * LC], ones_b[:],
                            probs[h][:, kc, 0:LC],
                            start=(kc == 0), stop=(kc == 1))
                rsum = statpool.tile([1, H * LC], F32, tag="rsum", bufs=2)
                nc.vector.reciprocal(rsum[:], ps_sum[0:1, 0:H * LC])
                rsb = statpool.tile([1, H * LC], BF16, tag="rsumb", bufs=2)
                nc.vector.tensor_copy(rsb[:], rsum[:])
                rs_ps = ppbc.tile([128, 512], F32, tag="bcast", name="rs_ps")
                nc.tensor.matmul(rs_ps[0:DH, :], ones_row[0:1, 0:DH],
                                 rsb[:], start=True, stop=True)
                rs_bc = tmppool.tile([DH, H, 3, LC], F32, tag="rs_bc", bufs=1)
                nc.vector.tensor_copy(
                    rs_bc[:, :, 0, :],
                    rs_ps[0:DH, :].rearrange("p (h q) -> p h q", q=LC))
                for h in range(H):
                    hh, g = h % 2, h // 2
                    ps_o = pp.tile([128, 2, W3], F32, tag="mm_out", name="ps_o")
                    for kc in range(2):
                        nc.tensor.matmul(
                            ps_o[0:DH, 0, 0:LC],
                            V3[kc][:, h * DH:(h + 1) * DH],
                            probs[h][:, kc, 0:LC],
                            start=(kc == 0), stop=(kc == 1))
                    nc.vector.tensor_mul(
                        oT[g][hh * DH:(hh + 1) * DH, :], ps_o[0:DH, 0, 0:LC],
                        rs_bc[:, h, 0, :])

            cur = load_layer(0)
            nxt = load_layer(1)
            hob = None
            for s in range(SEGS):
                lA, lB = 2 * s, 2 * s + 1
                w_qkv, w_proj, vb = cur
                mA = mpool.tile([128, H, 3, W3], BF16, tag="mA", name="mA")
                nc.sync.dma_start(mA[:], maskA[s])
                mB = mpool.tile([128, H, 2, LC], BF16, tag="mB", name="mB")
                nc.sync.dma_start(mB[:], maskB[s])

                hwb = bigpool.tile([128, DT, W5], BF16, tag="hwb")
                if s > 0:
                    # gather h_{lA-1} for the 5-tile kv range from ag_out
                    ag_out = ag_out_prev
                    for g in range(DT):
                        nc.sync.dma_start(
                            hwb[:, g, :].rearrange("p (r t) -> p r t", t=LC),
                            ag_out[bass.ds(k0v, 5),
                                   g * 128 * LC:(g + 1) * 128 * LC]
                            .rearrange("r (p t) -> p r t", t=LC))
                    nc.vector.tensor_copy(hwork[:], hwb[:])
                    # own tile stays f32-exact (and bf16 shadow from hob)
                    nc.sync.dma_start(
                        hwork[:, :, bass.ds(oiv * LC, LC)], hT_own[:])
                    nc.sync.dma_start(
                        hwb[:, :, bass.ds(oiv * LC, LC)], hob[:])
                else:
                    nc.vector.tensor_copy(hwb[:], hwork[:])

                # ---- layer A (3-tile working range, 5-tile kv range) ----
                x5 = bigpool.tile([128, DT, P5], BF16, tag="x5")
                nc.vector.memset(x5[:, :, W5:P5], 0.0)
                layernorm(hwork, 0, hwb, x5, 0, W3, "a")
                layernorm(hwork, W3, hwb, x5, W3, 2 * LC, "b")

                # hA = h values of the a-range (f32), own tile exact
                hA = bigpool.tile([128, DT, W3], F32, tag="hA")
                nc.sync.dma_start(hA[:], hwork[:, :, bass.ds(arv * LC, W3)])
                # xq = x~ of the a-range
                xq = actpool.tile([128, DT, W3], BF16, tag="xq")
                nc.sync.dma_start(xq[:], x5[:, :, bass.ds(arv * LC, W3)])

                # K for 5(+1 pad) tiles, feature-major per head-pair
                K5 = []
                for g in range(DT):
                    ps = ppv.tile([128, D], F32, tag="ps_v", name="ps_k5")
                    for h3 in range(3):
                        for f in range(DT):
                            nc.tensor.matmul(
                                ps[:, h3 * 128:(h3 + 1) * 128],
                                w_qkv[:, f * 1536 + 512 + g * 128:
                                      f * 1536 + 512 + (g + 1) * 128],
                                x5[:, f, h3 * 128:(h3 + 1) * 128],
                                start=(f == 0), stop=(f == DT - 1))
                    K5_g = actpool.tile([128, P5], BF16, tag=f"K5{g}",
                                        name="K5_g")
                    nc.vector.tensor_scalar_add(
                        K5_g[:], ps[:, 0:P5],
                        pbc(lA, PB_QKVB + DT + g))
                    K5.append(K5_g)
                # V for 3 chunks of 128 tokens, token-major
                V5 = []
                for kc in range(3):
                    ps_v = ppv.tile([128, D], F32, tag="ps_v")
                    for f in range(DT):
                        nc.tensor.matmul(
                            ps_v[:], x5[:, f, kc * 128:(kc + 1) * 128],
                            w_qkv[:, f * 1536 + 1024:f * 1536 + 1536],
                            start=(f == 0), stop=False)
                    nc.tensor.matmul(ps_v[:], ones_row[:], vb[:],
                                     start=False, stop=True)
                    V5_kc = actpool.tile([128, D], BF16, tag=f"V5{kc}",
                                         name="V5_kc")
                    nc.vector.tensor_copy(V5_kc[:], ps_v[:])
                    V5.append(V5_kc)

                # q for the 3 a-tiles
                qT = []
                for g in range(DT):
                    ps = pp.tile([128, 2, W3], F32, tag="mm_out")
                    for f in range(DT):
                        nc.tensor.matmul(
                            ps[:, 0, :],
                            w_qkv[:, f * 1536 + g * 128:f * 1536 + (g + 1) * 128],
                            xq[:, f, :], start=(f == 0), stop=(f == DT - 1))
                    qT_g = actpool.tile([128, W3], BF16, tag=f"qT{g}",
                                        name="qT_g")
                    nc.vector.tensor_scalar_add(
                        qT_g[:], ps[:, 0, :], pbc(lA, PB_QKVB + g))
                    qT.append(qT_g)

                oT = [actpool.tile([128, W3], BF16, tag=f"oT{g}", name="oT_g")
                      for g in range(DT)]
                attention_A(K5, V5, qT, mA, oT)

                # proj + residual into hA
                for t in range(DT):
                    ps = pp.tile([128, 2, W3], F32, tag="mm_out")
                    for f in range(DT):
                        nc.tensor.matmul(
                            ps[:, 0, :],
                            w_proj[:, f * 512 + t * 128:f * 512 + (t + 1) * 128],
                            oT[f][:], start=(f == 0), stop=(f == DT - 1))
                    nc.vector.scalar_tensor_tensor(
                        hA[:, t, :], ps[:, 0, :], pbc(lA, PB_PROJB + t),
                        hA[:, t, :], op0=mybir.AluOpType.add,
                        op1=mybir.AluOpType.add)

                # LN2 + FFN on the 3 a-tiles
                hAb = bigpool.tile([128, DT, W3], BF16, tag="hAb")
                nc.gpsimd.tensor_copy(hAb[:], hA[:])
                zA = bigpool.tile([128, DT, W3], BF16, tag="zA")
                layernorm(hA, 0, hAb, zA, 0, W3, "a")
                z1 = bigpool.tile([128, FT, W3], BF16, tag="z1")
                for ch in range(4):
                    wch = load_ff1_chunk(lA, ch)
                    for tl in range(4):
                        t = ch * 4 + tl
                        ps = pp.tile([128, 2, W3], F32, tag="mm_out",
                                     name="ps_ff1")
                        for f in range(DT):
                            nc.tensor.matmul(
                                ps[:, 0, :],
                                wch[:, tl * 512 + f * 128:
                                    tl * 512 + (f + 1) * 128],
                                zA[:, f, :], start=(f == 0),
                                stop=(f == DT - 1))
                        nc.scalar.activation(
                            z1[:, t, :], ps[:, 0, :],
                            mybir.ActivationFunctionType.Gelu,
                            bias=pb_all[:, lA, PB_FF1B + t:PB_FF1B + t + 1],
                            scale=1.0)
                for t in range(DT):
                    wch = load_ff2_chunk(lA, t)
                    ps = pp.tile([128, 2, W3], F32, tag="mm_out")
                    for g in range(FT):
                        nc.tensor.matmul(
                            ps[:, 0, :], wch[:, g * 128:(g + 1) * 128],
                            z1[:, g, :], start=(g == 0), stop=(g == FT - 1))
                    nc.vector.scalar_tensor_tensor(
                        hA[:, t, :], ps[:, 0, :], pbc(lA, PB_FF2B + t),
                        hA[:, t, :], op0=mybir.AluOpType.add,
                        op1=mybir.AluOpType.add)

                # ---- layer B (own tile; window = the 3 a-tiles) ----
                w_qkvB, w_projB, vbB = nxt
                if s + 1 < SEGS:
                    cur = load_layer(lA + 2)

                hAb2 = bigpool.tile([128, DT, W3], BF16, tag="hAb2")
                nc.gpsimd.tensor_copy(hAb2[:], hA[:])
                x3 = bigpool.tile([128, DT, P3], BF16, tag="x3")
                nc.vector.memset(x3[:, :, W3:P3], 0.0)
                layernorm(hA, 0, hAb2, x3, 0, W3, "a")

                own_hA = actpool.tile([128, DT, LC], F32, tag="own_hA")
                nc.sync.dma_start(own_hA[:], hA[:, :, bass.ds(oiAv * LC, LC)])
                own_xq = actpool.tile([128, DT, LC], BF16, tag="own_xq")
                nc.sync.dma_start(own_xq[:], x3[:, :, bass.ds(oiAv * LC, LC)])

                K3 = []
                for g in range(DT):
                    ps = ppv.tile([128, D], F32, tag="ps_v", name="ps_k3")
                    for h2 in range(2):
                        for f in range(DT):
                            nc.tensor.matmul(
                                ps[:, h2 * 128:(h2 + 1) * 128],
                                w_qkvB[:, f * 1536 + 512 + g * 128:
                                       f * 1536 + 512 + (g + 1) * 128],
                                x3[:, f, h2 * 128:(h2 + 1) * 128],
                                start=(f == 0), stop=(f == DT - 1))
                    K3_g = actpool.tile([128, P3], BF16, tag=f"K3{g}",
                                        name="K3_g")
                    nc.vector.tensor_scalar_add(
                        K3_g[:], ps[:, 0:P3], pbc(lB, PB_QKVB + DT + g))
                    K3.append(K3_g)
                V3 = []
                for kc in range(2):
                    ps_v = ppv.tile([128, D], F32, tag="ps_v")
                    for f in range(DT):
                        nc.tensor.matmul(
                            ps_v[:], x3[:, f, kc * 128:(kc + 1) * 128],
                            w_qkvB[:, f * 1536 + 1024:f * 1536 + 1536],
                            start=(f == 0), stop=False)
                    nc.tensor.matmul(ps_v[:], ones_row[:], vbB[:],
                                     start=False, stop=True)
                    V3_kc = actpool.tile([128, D], BF16, tag=f"V3{kc}",
                                         name="V3_kc")
                    nc.vector.tensor_copy(V3_kc[:], ps_v[:])
                    V3.append(V3_kc)

                qTB = []
                for g in range(DT):
                    ps = pp.tile([128, 2, W3], F32, tag="mm_out")
                    for f in range(DT):
                        nc.tensor.matmul(
                            ps[:, 0, 0:LC],
                            w_qkvB[:, f * 1536 + g * 128:f * 1536 + (g + 1) * 128],
                            own_xq[:, f, :], start=(f == 0), stop=(f == DT - 1))
                    qTB_g = actpool.tile([128, LC], BF16, tag=f"qTB{g}",
                                         name="qTB_g")
                    nc.vector.tensor_scalar_add(
                        qTB_g[:], ps[:, 0, 0:LC], pbc(lB, PB_QKVB + g))
                    qTB.append(qTB_g)

                oTB = [actpool.tile([128, LC], BF16, tag=f"oTB{g}",
                                    name="oTB_g") for g in range(DT)]
                attention_B(K3, V3, qTB, mB, oTB)

                # proj + residual: hT_own = own_hA + proj(oTB) + b
                for t in range(DT):
                    ps = pp.tile([128, 2, W3], F32, tag="mm_out")
                    for f in range(DT):
                        nc.tensor.matmul(
                            ps[:, 0, 0:LC],
                            w_projB[:, f * 512 + t * 128:f * 512 + (t + 1) * 128],
                            oTB[f][:], start=(f == 0), stop=(f == DT - 1))
                    nc.vector.scalar_tensor_tensor(
                        hT_own[:, t, :], ps[:, 0, 0:LC], pbc(lB, PB_PROJB + t),
                        own_hA[:, t, :], op0=mybir.AluOpType.add,
                        op1=mybir.AluOpType.add)

                hTb = actpool.tile([128, DT, LC], BF16, tag="hTb")
                nc.gpsimd.tensor_copy(hTb[:], hT_own[:])
                zB = bigpool.tile([128, DT, LC], BF16, tag="zB")
                layernorm(hT_own, 0, hTb, zB, 0, LC, "c")
                z1B = bigpool.tile([128, FT, LC], BF16, tag="z1B")
                for ch in range(4):
                    wch = load_ff1_chunk(lB, ch)
                    for tt in range(4):
                        t = ch * 4 + tt
                        ps = pp.tile([128, 2, W3], F32, tag="mm_out",
                                     name="ps_ff1b")
                        for f in range(DT):
                            nc.tensor.matmul(
                                ps[:, 0, 0:LC],
                                wch[:, tt * 512 + f * 128:
                                    tt * 512 + (f + 1) * 128],
                                zB[:, f, :], start=(f == 0),
                                stop=(f == DT - 1))
                        nc.scalar.activation(
                            z1B[:, t, :], ps[:, 0, 0:LC],
                            mybir.ActivationFunctionType.Gelu,
                            bias=pb_all[:, lB, PB_FF1B + t:PB_FF1B + t + 1],
                            scale=1.0)
                for t in range(DT):
                    wch = load_ff2_chunk(lB, t)
                    ps = pp.tile([128, 2, W3], F32, tag="mm_out")
                    for g in range(FT):
                        nc.tensor.matmul(
                            ps[:, 0, 0:LC], wch[:, g * 128:(g + 1) * 128],
                            z1B[:, g, :], start=(g == 0), stop=(g == FT - 1))
                    nc.vector.scalar_tensor_tensor(
                        hT_own[:, t, :], ps[:, 0, 0:LC], pbc(lB, PB_FF2B + t),
                        hT_own[:, t, :], op0=mybir.AluOpType.add,
                        op1=mybir.AluOpType.add)

                # ---- exchange h_{lB} (own tile) for the next segment ----
                if s + 1 < SEGS:
                    nxt = load_layer(lA + 3)
                    hob = actpool.tile([128, DT, LC], BF16, tag="hob")
                    nc.gpsimd.tensor_copy(hob[:], hT_own[:])
                    ag_in = agdram.tile([D * LC], BF16, tag="ag_in")
                    ag_out_prev = agdram.tile([NC, D * LC], BF16, tag="ag_out",
                                              addr_space="Shared")
                    nc.sync.dma_start(
                        ag_in[:].rearrange("(f p t) -> p f t", p=128, t=LC),
                        hob[:])
                    nc.gpsimd.collective_compute(
                        "AllGather", mybir.AluOpType.bypass,
                        ins=[ag_in[:].opt()], outs=[ag_out_prev[:].opt()],
                        replica_groups=[list(range(NC))])

            # ---- output projection: y.T = tanh(out_w @ hT_own + out_b) ----
            hb = actpool.tile([128, DT, LC], BF16, tag="hb")
            nc.vector.tensor_copy(hb[:], hT_own[:])
            ps_y = pp.tile([128, 2, W3], F32, tag="mm_out", name="ps_y")
            for f in range(DT):
                nc.tensor.matmul(ps_y[0:PATCH, 0, 0:LC],
                                 wout_s[:, f * PATCH:(f + 1) * PATCH],
                                 hb[:, f, :], start=(f == 0), stop=(f == DT - 1))
            y_sb = actpool.tile([PATCH, LC], F32, tag="y_sb")
            nc.scalar.activation(y_sb[:], ps_y[0:PATCH, 0, 0:LC],
                                 mybir.ActivationFunctionType.Tanh,
                                 bias=outb_s[:, 0:1], scale=1.0)
            nc.sync.dma_start(yT[:], y_sb[:])

    nc.finalize()
    return nc


def _prep_inputs(inputs):
    """Host-side: pack full fp32 inputs into per-core in_maps."""
    I = {k: np.asarray(v, np.float32) for k, v in inputs.items()}

    scale = np.float32(DH ** -0.5)
    qkv_w = I["qkv_w"].copy()          # [LAYERS, 3D, D]
    qkv_b = I["qkv_b"].copy()          # [LAYERS, 3D]
    ff1_w = I["ff1_w"].copy()          # [LAYERS, DFF, D]
    ff1_b = I["ff1_b"].copy()          # [LAYERS, DFF]
    for l in range(LAYERS):
        qkv_b[l] += qkv_w[l] @ I["ln1_b"][l]
        qkv_w[l] *= I["ln1_g"][l][None, :]
        ff1_b[l] += ff1_w[l] @ I["ln2_b"][l]
        ff1_w[l] *= I["ln2_g"][l][None, :]
    qkv_w[:, :D] *= scale
    qkv_b[:, :D] *= scale

    def part_major(m):
        X = m.shape[0] // 128
        return np.ascontiguousarray(
            m.reshape(X, 128, m.shape[1]).transpose(1, 0, 2).reshape(128, -1))

    wblob = np.empty((LAYERS, 128, WCOLS), BF)
    pblob = np.zeros((LAYERS, 128, PCOLS), np.float32)
    for l in range(LAYERS):
        qkvT = np.ascontiguousarray(qkv_w[l].T)          # [D, 3D]
        projT = np.ascontiguousarray(I["proj_w"][l].T)   # [D, D]
        ff1T = np.ascontiguousarray(ff1_w[l].T)          # [D, DFF]
        ff2T = np.ascontiguousarray(I["ff2_w"][l].T)     # [DFF, D]
        wblob[l, :, OFF_QKV:OFF_PROJ] = part_major(qkvT).astype(BF)
        wblob[l, :, OFF_PROJ:OFF_FF1] = part_major(projT).astype(BF)
        # FF regions fo-major so they stream in per-fo chunks
        wblob[l, :, OFF_FF1:OFF_FF2] = (
            part_major(ff1T).reshape(128, 4, 16, 128)
            .transpose(0, 2, 1, 3).reshape(128, 8192).astype(BF))
        wblob[l, :, OFF_FF2:WCOLS] = (
            part_major(ff2T).reshape(128, 16, 4, 128)
            .transpose(0, 2, 1, 3).reshape(128, 8192).astype(BF))
        pblob[l, :, PB_QKVB:PB_QKVB + 12] = qkv_b[l].reshape(12, 128).T
        pblob[l, :, PB_PROJB:PB_PROJB + 4] = I["proj_b"][l].reshape(4, 128).T
        pblob[l, :, PB_FF1B:PB_FF1B + 16] = ff1_b[l].reshape(16, 128).T
        pblob[l, :, PB_FF2B:PB_FF2B + 4] = I["ff2_b"][l].reshape(4, 128).T
    vbias = np.ascontiguousarray(qkv_b[:, 2 * D:3 * D]).astype(BF)
    fbrow = ff1_b.astype(BF)

    # attention bias+mask table over global (key, query) pairs
    i = np.arange(L)
    ni = np.clip(i - K // 2, 0, L - K)
    k_idx = np.arange(L)[:, None]
    in_win = (k_idx >= ni[None, :]) & (k_idx < (ni + K)[None, :])
    rel = np.clip(k_idx - i[None, :] + (K - 1), 0, 2 * K - 2)
    rpb = I["rpb"]                                       # [LAYERS, H, 2K-1]
    B_full = np.where(in_win[None, None], rpb[:, :, rel],
                      np.float32(NEG)).astype(np.float32)  # [LAYERS,H,L,L]

    w_in_T = np.ascontiguousarray(I["in_w"].T).astype(BF)
    inb = np.ascontiguousarray(I["in_b"].reshape(DT, 128).T)
    out_wT = np.ascontiguousarray(I["out_w"].T)
    w_out = part_major(out_wT).astype(BF)
    outb = np.ascontiguousarray(I["out_b"].reshape(PATCH, 1))

    x_tok = I["x"].reshape(L, PATCH)                     # [L, PATCH]

    in_maps = []
    for c in range(NC):
        k0 = min(max(c - 2, 0), 3)
        a0 = min(max(c - 1, 0), 5)
        xT_c = np.ascontiguousarray(x_tok[c * LC:(c + 1) * LC].T).astype(BF)
        x5_c = np.ascontiguousarray(
            x_tok[k0 * LC:(k0 + 5) * LC].T).astype(BF)   # [PATCH, 320]

        # masks packed partition-major: mA[s, p, h, kc, j*64+qi]
        mA = np.full((SEGS, 128, H, 3, W3), NEG, np.float32)
        mB = np.full((SEGS, 128, H, 2, LC), NEG, np.float32)
        for s in range(SEGS):
            blkA = B_full[2 * s, :, k0 * LC:k0 * LC + W5, :]   # [H, 320, L]
            for j in range(3):
                t = a0 + j
                sl = blkA[:, :, t * LC:(t + 1) * LC]           # [H, 320, LC]
                for kc in range(3):
                    lo, hi = kc * 128, min((kc + 1) * 128, W5)
                    # [H, rows, LC] -> [rows, H, LC]
                    mA[s, 0:hi - lo, :, kc, j * LC:(j + 1) * LC] = (
                        sl[:, lo:hi].transpose(1, 0, 2))
            blkB = B_full[2 * s + 1, :, a0 * LC:a0 * LC + W3,
                          c * LC:(c + 1) * LC]                 # [H, 192, LC]
            for kc in range(2):
                lo, hi = kc * 128, min((kc + 1) * 128, W3)
                mB[s, 0:hi - lo, :, kc, :] = blkB[:, lo:hi].transpose(1, 0, 2)

        mA = np.exp(mA)
        mB = np.exp(mB)
        in_maps.append({
            "xT": xT_c,
            "x5in": x5_c,
            "w_in_T": w_in_T,
            "inb": inb,
            "wblob": wblob,
            "pblob": pblob,
            "vbias": vbias,
            "fbrow": fbrow,
            "maskA": mA.astype(BF),
            "maskB": mB.astype(BF),
            "w_out": w_out,
            "outb": outb,
        })
    return in_maps


def kernel(**inputs):
    if "nc" not in _BUILD_CACHE:
        _BUILD_CACHE["nc"] = _build()
    nc = _BUILD_CACHE["nc"]
    in_maps = _prep_inputs(inputs)
    res = run_bass_kernel_spmd(nc, in_maps, core_ids=list(range(NC)))
    y = np.empty((1, 1, L * PATCH), np.float32)
    for c in range(NC):
        yT_c = res.results[c]["yT"]                      # [PATCH, LC]
        y[0, 0, c * LC * PATCH:(c + 1) * LC * PATCH] = yT_c.T.reshape(-1)
    return y


# revision 34
# speedup vs baseline: 1.0643x; 1.0479x over previous
"""Trainium2 Bass kernel for nn_AudioTransformer (neighborhood-attention transformer).

Strategy: sequence-parallel over 8 NeuronCores (64 tokens/core) with BATCHED
halo exchange: layers run in 4 segments of 2; per segment each core
redundantly computes a 3-tile working range (own tile +-1) for the first
layer (A) and just its own tile for the second (B), consuming a 5-tile K/V
range gathered once per segment. Only 3 AllGathers total (after layers 1, 3,
5); the full input is free. The own-token residual stream stays fp32 end to
end; only halo copies of h cross cores (bf16) and only affect attention
values.

Engine budget: matmuls on PE with bf16 operands wherever a reduction allows
(fp32 matmuls below 256 free-columns run at 1/4 rate); attention mask-adds on
the otherwise idle Pool engine; FF1/FF2 weight chunks stream on the ACT/DVE
DMA queues so the SP queue only carries QKV/proj weights and dynamic
gathers; masks are packed partition-major on the host so their DMA runs at
full element width.
"""

import numpy as np
import ml_dtypes

import concourse.bass as bass
import concourse.mybir as mybir
import concourse.tile as tile
from concourse.tile import add_dep_helper
from concourse import bacc
from concourse.bass_utils import run_bass_kernel_spmd


def _install_act_table_filter():
    """Make the act-table chooser resolve Ln/Exp/Identity/Copy only via the
    natural_log_exp_and_others set so each layer needs just 2 LUT swaps
    (to gelu_and_others and back) instead of 5."""
    import concourse.bacc as _bacc_mod
    if getattr(_bacc_mod, "_ant_act_filter", False):
        return
    _orig = _bacc_mod.get_activation_tables
    A = mybir.ActivationFunctionType
    movable = {A.Ln, A.Exp, A.Identity, A.Copy}

    def _filtered(arch):
        t = _orig(arch)
        out = {}
        for name, funcs in t.items():
            if name == "natural_log_exp_and_others":
                out[name] = set(funcs)
            else:
                out[name] = set(funcs) - movable
        return out

    _bacc_mod.get_activation_tables = _filtered
    _bacc_mod._ant_act_filter = True

BF = ml_dtypes.bfloat16
F32 = mybir.dt.float32
BF16 = mybir.dt.bfloat16

NC = 8          # cores
L = 512         # total tokens
LC = L // NC    # tokens per core/tile = 64
D = 512         # model dim
DT = D // 128   # 4 feature tiles
H = 8           # heads
DH = 64         # head dim
DFF = 2048      # ff dim
FT = DFF // 128  # 16 ff tiles
PATCH = 32
LAYERS = 8
SEGS = LAYERS // 2
K = 127         # neighborhood size
NEG = -60.0     # out-of-window logit bias (exp(-60+2) == 0 in fp32/bf16)

W5 = 5 * LC     # kv-range width (320)
P5 = 6 * LC     # padded kv width (384, 3 chunks of 128)
W3 = 3 * LC     # a-range width (192)
P3 = 4 * LC     # padded a-range width (256, 2 chunks of 128)

# wblob column offsets (per 128-row partition, bf16)
OFF_QKV = 0            # 4 fi-tiles x 1536
OFF_PROJ = 6144        # 4 fi-tiles x 512
OFF_FF1 = 8192         # fo-major [fo:16][fi:4][128]
OFF_FF2 = 16384        # fo-major [fo:4][g:16][128]
WCOLS = 24576

# pblob columns (f32)
PB_QKVB = 0    # 12
PB_PROJB = 12  # 4
PB_FF1B = 16   # 16
PB_FF2B = 32   # 4
PCOLS = 52

_BUILD_CACHE = {}


def _build():
    """Build + finalize the SPMD Bass graph (same graph on all 8 cores)."""
    _install_act_table_filter()
    nc = bacc.Bacc(None, target_bir_lowering=False)

    # ---- DRAM parameters (per-core inputs) ----
    xT = nc.dram_tensor("xT", [PATCH, LC], BF16, kind="ExternalInput")
    x5in = nc.dram_tensor("x5in", [PATCH, W5], BF16, kind="ExternalInput")
    w_in_T = nc.dram_tensor("w_in_T", [PATCH, D], BF16, kind="ExternalInput")
    inb = nc.dram_tensor("inb", [128, DT], F32, kind="ExternalInput")
    wblob = nc.dram_tensor("wblob", [LAYERS, 128, WCOLS], BF16, kind="ExternalInput")
    pblob = nc.dram_tensor("pblob", [LAYERS, 128, PCOLS], F32, kind="ExternalInput")
    vbias = nc.dram_tensor("vbias", [LAYERS, D], BF16, kind="ExternalInput")
    fbrow = nc.dram_tensor("fbrow", [LAYERS, DFF], BF16, kind="ExternalInput")
    maskA = nc.dram_tensor("maskA", [SEGS, 128, H, 3, W3], BF16,
                           kind="ExternalInput")
    maskB = nc.dram_tensor("maskB", [SEGS, 128, H, 2, LC], BF16,
                           kind="ExternalInput")
    w_out = nc.dram_tensor("w_out", [128, 128], BF16, kind="ExternalInput")
    outb = nc.dram_tensor("outb", [PATCH, 1], F32, kind="ExternalInput")
    yT = nc.dram_tensor("yT", [PATCH, LC], F32, kind="ExternalOutput")

    with tile.TileContext(nc) as tc:
        with (
            tc.tile_pool(name="singles", bufs=1) as singles,
            tc.tile_pool(name="wpool", bufs=2) as wpool,
            tc.tile_pool(name="mpool", bufs=1) as mpool,
            tc.tile_pool(name="bigpool", bufs=1) as bigpool,
            tc.tile_pool(name="actpool", bufs=2) as actpool,
            tc.tile_pool(name="tmppool", bufs=2) as tmppool,
            tc.tile_pool(name="statpool", bufs=1) as statpool,
            tc.tile_pool(name="agdram", bufs=2, space="DRAM") as agdram,
            # PSUM: 8 banks. pp:mm_out(3) + pp_ln(1) + ppv(1) + ppbc(1)
            #  + ppatt:ps_l(2) = 8
            tc.tile_pool(name="pp", bufs=3, space="PSUM") as pp,
            tc.tile_pool(name="pp_ln", bufs=1, space="PSUM") as pp_ln,
            tc.tile_pool(name="ppv", bufs=1, space="PSUM") as ppv,
            tc.tile_pool(name="ppatt", bufs=2, space="PSUM") as ppatt,
            tc.tile_pool(name="ppbc", bufs=1, space="PSUM") as ppbc,
        ):
            # persistent tiles
            hT_own = singles.tile([128, DT, LC], F32)   # own residual, f32
            hwork = singles.tile([128, DT, W5], F32)    # segment kv-range h
            ones_f = singles.tile([128, 1], F32)
            ones_b = singles.tile([128, 1], BF16)
            ones_row = singles.tile([1, 128], BF16)
            xin = singles.tile([PATCH, LC], BF16)
            x5s = singles.tile([PATCH, W5], BF16)
            win = singles.tile([PATCH, D], BF16)
            inb_s = singles.tile([128, DT], F32)
            wout_s = singles.tile([128, 128], BF16)
            outb_s = singles.tile([PATCH, 1], F32)
            pb_all = singles.tile([128, LAYERS, PCOLS], F32)

            nc.vector.memset(ones_f[:], 1.0)
            nc.vector.memset(ones_b[:], 1.0)
            nc.vector.memset(ones_row[:], 1.0)
            nc.sync.dma_start(xin[:], xT[:])
            nc.sync.dma_start(x5s[:], x5in[:])
            nc.sync.dma_start(win[:], w_in_T[:])
            nc.sync.dma_start(inb_s[:], inb[:])
            nc.sync.dma_start(wout_s[:], w_out[:])
            nc.sync.dma_start(outb_s[:], outb[:])
            nc.sync.dma_start(pb_all[:], pblob[:].rearrange("l p c -> p l c"))

            # per-core clip offsets as branch-free register arithmetic
            rank = nc.sync.partition_id()
            k0v = ((rank - 2) * ((rank >= 3) & (rank <= 5))
                   + 3 * (rank >= 6))                 # clip(c-2, 0, 3)
            a0v = ((rank - 1) * ((rank >= 1) & (rank <= 6))
                   + 5 * (rank >= 7))                 # clip(c-1, 0, 5)
            arv = a0v - k0v                           # a-range offset in kv range
            oiAv = rank - a0v                         # own tile within a-range
            oiv = rank - k0v                          # own tile within kv range

            def pbc(l, col):
                return pb_all[:, l, col:col + 1]

            def layernorm(srcT, sc0, srcbT, dstT, dc0, ncols, key):
                """normalize(srcT[:,:,sc0:+n]) -> dstT[:,:,dc0:+n] (bf16).
                srcbT: bf16 shadow tile of srcT (same columns) used for the
                PE reductions so they run at full bf16 rate."""
                src = srcT[:, :, sc0:sc0 + ncols]
                sqb = tmppool.tile([128, DT, ncols], BF16, tag=f"ln_sq{key}",
                                   bufs=1)
                nc.vector.tensor_mul(sqb[:], srcbT[:, :, sc0:sc0 + ncols],
                                     srcbT[:, :, sc0:sc0 + ncols])
                ps_s = pp_ln.tile([1, 512], F32, tag="sums", name="ps_s")
                for f in range(DT):
                    nc.tensor.matmul(ps_s[0:1, 0:ncols], ones_b[:],
                                     srcbT[:, f, sc0:sc0 + ncols],
                                     start=(f == 0), stop=(f == DT - 1))
                for f in range(DT):
                    nc.tensor.matmul(ps_s[0:1, 256:256 + ncols], ones_b[:],
                                     sqb[:, f, :],
                                     start=(f == 0), stop=(f == DT - 1))
                st = statpool.tile([1, 2 * ncols], F32, tag=f"ln_st{key}")
                nc.vector.tensor_scalar_mul(st[0:1, 0:ncols],
                                            ps_s[0:1, 0:ncols], 1.0 / D)
                m2 = statpool.tile([1, ncols], F32, tag=f"ln_m2{key}")
                nc.vector.tensor_mul(m2[:], st[0:1, 0:ncols], st[0:1, 0:ncols])
                nc.vector.tensor_scalar_add(m2[:], m2[:], -1e-5)
                var = statpool.tile([1, ncols], F32, tag=f"ln_var{key}")
                nc.vector.scalar_tensor_tensor(
                    var[:], ps_s[0:1, 256:256 + ncols], 1.0 / D, m2[:],
                    op0=mybir.AluOpType.mult, op1=mybir.AluOpType.subtract)
                # rstd = exp(-0.5*ln(var)) -- keeps ACT in the Ln/Exp func set
                sd = statpool.tile([1, ncols], F32, tag=f"ln_sd{key}")
                nc.scalar.activation(sd[:], var[:],
                                     mybir.ActivationFunctionType.Ln)
                nc.scalar.activation(st[0:1, ncols:2 * ncols], sd[:],
                                     mybir.ActivationFunctionType.Exp,
                                     scale=-0.5)
                stb = statpool.tile([1, 2 * ncols], BF16, tag=f"ln_stb{key}")
                nc.vector.tensor_copy(stb[:], st[:])
                # broadcast (mean, rstd) across partitions via K=1 matmul
                bc = ppbc.tile([128, 512], F32, tag="bcast", name="bc")
                nc.tensor.matmul(bc[:, 0:ncols], ones_row[:],
                                 stb[0:1, 0:ncols], start=True, stop=True)
                nc.tensor.matmul(bc[:, 256:256 + ncols], ones_row[:],
                                 stb[0:1, ncols:2 * ncols],
                                 start=True, stop=True)
                mean_w = bc[:, 0:ncols].unsqueeze(1).to_broadcast(
                    [128, DT, ncols])
                rstd_w = bc[:, 256:256 + ncols].unsqueeze(1).to_broadcast(
                    [128, DT, ncols])
                t0 = tmppool.tile([128, DT, ncols], F32, tag=f"ln_t0{key}",
                                  bufs=1)
                nc.vector.tensor_sub(t0[:], src, mean_w)
                nc.vector.tensor_mul(dstT[:, :, dc0:dc0 + ncols], t0[:],
                                     rstd_w)

            # ---- input projection ----
            for t in range(DT):
                ps = pp.tile([128, 2, W3], F32, tag="mm_out")
                nc.tensor.matmul(ps[:, 0, 0:LC], win[:, t * 128:(t + 1) * 128],
                                 xin[:], start=True, stop=True)
                nc.vector.tensor_scalar_add(hT_own[:, t, :], ps[:, 0, 0:LC],
                                            inb_s[:, t:t + 1])
            for t in range(DT):
                ps = pp.tile([128, 2, W3], F32, tag="mm_out")
                nc.tensor.matmul(ps[:, 0, 0:W3], win[:, t * 128:(t + 1) * 128],
                                 x5s[:, 0:W3], start=True, stop=True)
                nc.vector.tensor_scalar_add(hwork[:, t, 0:W3], ps[:, 0, 0:W3],
                                            inb_s[:, t:t + 1])
                ps2 = pp.tile([128, 2, W3], F32, tag="mm_out")
                nc.tensor.matmul(ps2[:, 0, 0:2 * LC],
                                 win[:, t * 128:(t + 1) * 128],
                                 x5s[:, W3:W5], start=True, stop=True)
                nc.vector.tensor_scalar_add(hwork[:, t, W3:W5],
                                            ps2[:, 0, 0:2 * LC],
                                            inb_s[:, t:t + 1])

            def load_layer(l):
                w_qkv = wpool.tile([128, 6144], BF16, tag="w_qkv", name="w_qkv")
                w_proj = wpool.tile([128, 2048], BF16, tag="w_proj",
                                    name="w_proj")
                vb = wpool.tile([1, D], BF16, tag="vb", name="vb")
                nc.sync.dma_start(w_qkv[:], wblob[l, :, OFF_QKV:OFF_PROJ])
                nc.sync.dma_start(w_proj[:], wblob[l, :, OFF_PROJ:OFF_FF1])
                nc.sync.dma_start(vb[:], vbias[l].unsqueeze(0))
                return w_qkv, w_proj, vb

            def load_ff1_chunk(l, ch):
                """4 fo-tiles of FF1 weights: [tt:4][fi:4][128] columns."""
                w = wpool.tile([128, 2048], BF16, tag="ff1c", name="ff1c",
                               bufs=3)
                nc.sync.dma_start(
                    w[:], wblob[l, :, OFF_FF1 + ch * 2048:
                                OFF_FF1 + (ch + 1) * 2048])
                return w

            def load_ff2_chunk(l, t):
                """One fo-tile of FF2 weights: [g:16][128] columns."""
                w = wpool.tile([128, 2048], BF16, tag="ff2c", name="ff2c",
                               bufs=3)
                nc.gpsimd.dma_start(
                    w[:], wblob[l, :, OFF_FF2 + t * 2048:
                                OFF_FF2 + (t + 1) * 2048])
                return w

            def attention_A(K5, V5, qT, mA, oT):
                """3 query tiles x 320-key window, chunked [128,128,64+pad]."""
                probs = []
                for h in range(H):
                    hh, g = h % 2, h // 2
                    ps12 = ppatt.tile([128, 2, W3], F32, tag="ps_l")
                    ps3p = ppv if h % 2 == 0 else ppbc
                    ps3 = ps3p.tile([128, D] if h % 2 == 0 else [128, 512],
                                    F32, tag="ps_v" if h % 2 == 0 else "bcast",
                                    name="ps3")
                    for kc in range(2):
                        nc.tensor.matmul(
                            ps12[:, kc, :],
                            K5[g][hh * DH:(hh + 1) * DH,
                                  kc * 128:(kc + 1) * 128],
                            qT[g][hh * DH:(hh + 1) * DH, :],
                            start=True, stop=True)
                    nc.tensor.matmul(
                        ps3[:, 0:W3],
                        K5[g][hh * DH:(hh + 1) * DH, 256:384],
                        qT[g][hh * DH:(hh + 1) * DH, :],
                        start=True, stop=True)
                    pe = tmppool.tile([128, 3, W3], BF16, tag="att_e",
                                      bufs=2)
                    nc.scalar.activation(pe[:, 0:2, :], ps12[:],
                                         mybir.ActivationFunctionType.Exp)
                    nc.scalar.activation(pe[:, 2, :], ps3[:, 0:W3],
                                         mybir.ActivationFunctionType.Exp)
                    probs_h = actpool.tile([128, 3, W3], BF16, tag=f"probs{h}",
                                           name="probs_h", bufs=1)
                    nc.vector.tensor_mul(probs_h[:], pe[:], mA[:, h, :, :])
                    probs.append(probs_h)
                # denominators per query tile
                rs_bc = tmppool.tile([DH, H, 3, LC], F32, tag="rs_bc", bufs=1)
                for j in range(3):
                    ps_sum = pp_ln.tile([1, 512], F32, tag="sums",
                                        name="ps_sum")
                    for h in range(H):
                        for kc in range(3):
                            nc.tensor.matmul(
                                ps_sum[0:1, h * LC:(h + 1) * LC], ones_b[:],
                                probs[h][:, kc, j * LC:(j + 1) * LC],
                                start=(kc == 0), stop=(kc == 2))
                    rsum = statpool.tile([1, H * LC], F32, tag="rsum", bufs=2)
                    nc.vector.reciprocal(rsum[:], ps_sum[0:1, 0:H * LC])
                    rsb = statpool.tile([1, H * LC], BF16, tag="rsumb", bufs=2)
                    nc.vector.tensor_copy(rsb[:], rsum[:])
                    rs_ps = ppbc.tile([128, 512], F32, tag="bcast",
                                      name="rs_ps")
                    nc.tensor.matmul(rs_ps[0:DH, :], ones_row[0:1, 0:DH],
                                     rsb[:], start=True, stop=True)
                    nc.vector.tensor_copy(
                        rs_bc[:, :, j, :],
                        rs_ps[0:DH, :].rearrange("p (h q) -> p h q", q=LC))
                # AV + scale, N=192 per head
                for h in range(H):
                    hh, g = h % 2, h // 2
                    ps_o = pp.tile([128, 2, W3], F32, tag="mm_out", name="ps_o")
                    for kc in range(3):
                        nc.tensor.matmul(
                            ps_o[0:DH, 0, :],
                            V5[kc][:, h * DH:(h + 1) * DH],
                            probs[h][:, kc, :],
                            start=(kc == 0), stop=(kc == 2))
                    nc.vector.tensor_mul(
                        oT[g][hh * DH:(hh + 1) * DH, :], ps_o[0:DH, 0, :],
                        rs_bc[:, h].rearrange("p j q -> p (j q)"))

            def attention_B(K3, V3, qT, mB, oT):
                """1 query tile x 192-key window, chunks [128, 64+pad]."""
                probs = []
                for h in range(H):
                    hh, g = h % 2, h // 2
                    ps12 = ppatt.tile([128, 2, W3], F32, tag="ps_l")
                    for kc in range(2):
                        nc.tensor.matmul(
                            ps12[:, kc, 0:LC],
                            K3[g][hh * DH:(hh + 1) * DH,
                                  kc * 128:(kc + 1) * 128],
                            qT[g][hh * DH:(hh + 1) * DH, :],
                            start=True, stop=True)
                    pe = tmppool.tile([128, 3, W3], BF16, tag="att_e",
                                      bufs=2)
                    nc.scalar.activation(pe[:, 0:2, 0:LC], ps12[:, :, 0:LC],
                                         mybir.ActivationFunctionType.Exp)
                    probs_h = actpool.tile([128, 3, W3], BF16, tag=f"probs{h}",
                                           name="probs_h", bufs=1)
                    nc.vector.tensor_mul(probs_h[:, 0:2, 0:LC],
                                         pe[:, 0:2, 0:LC], mB[:, h, :, :])
                    probs.append(probs_h)
                ps_sum = pp_ln.tile([1, 512], F32, tag="sums", name="ps_sum")
                for h in range(H):
                    for kc in range(2):
                        nc.tensor.matmul(
                            ps_sum[0:1, h * LC:(h + 1) * LC], ones_b[:],
                            probs[h][:, kc, 0:LC],
                            start=(kc == 0), stop=(kc == 1))
                rsum = statpool.tile([1, H * LC], F32, tag="rsum", bufs=2)
                nc.vector.reciprocal(rsum[:], ps_sum[0:1, 0:H * LC])
                rsb = statpool.tile([1, H * LC], BF16, tag="rsumb", bufs=2)
                nc.vector.tensor_copy(rsb[:], rsum[:])
                rs_ps = ppbc.tile([128, 512], F32, tag="bcast", name="rs_ps")
                nc.tensor.matmul(rs_ps[0:DH, :], ones_row[0:1, 0:DH],
                                 rsb[:], start=True, stop=True)
                rs_bc = tmppool.tile([DH, H, 3, LC], F32, tag="rs_bc", bufs=1)
                nc.vector.tensor_copy(
                    rs_bc[:, :, 0, :],
                    rs_ps[0:DH, :].rearrange("p (h q) -> p h q", q=LC))
                for h in range(H):
                    hh, g = h % 2, h // 2
                    ps_o = pp.tile([128, 2, W3], F32, tag="mm_out", name="ps_o")
                    for kc in range(2):
                        nc.tensor.matmul(
                            ps_o[0:DH, 0, 0:LC],
                            V3[kc][:, h * DH:(h + 1) * DH],
                            probs[h][:, kc, 0:LC],
                            start=(kc == 0), stop=(kc == 1))
                    nc.vector.tensor_mul(
                        oT[g][hh * DH:(hh + 1) * DH, :], ps_o[0:DH, 0, 0:LC],
                        rs_bc[:, h, 0, :])

            cur = load_layer(0)
            nxt = load_layer(1)
            hob = None
            for s in range(SEGS):
                lA, lB = 2 * s, 2 * s + 1
                w_qkv, w_proj, vb = cur
                mA = mpool.tile([128, H, 3, W3], BF16, tag="mA", name="mA")
                nc.sync.dma_start(mA[:], maskA[s])
                mB = mpool.tile([128, H, 2, LC], BF16, tag="mB", name="mB")
                nc.sync.dma_start(mB[:], maskB[s])

                hwb = bigpool.tile([128, DT, W5], BF16, tag="hwb")
                if s > 0:
                    # gather h_{lA-1} for the 5-tile kv range from ag_out
                    ag_out = ag_out_prev
                    for g in range(DT):
                        nc.sync.dma_start(
                            hwb[:, g, :].rearrange("p (r t) -> p r t", t=LC),
                            ag_out[bass.ds(k0v, 5),
                                   g * 128 * LC:(g + 1) * 128 * LC]
                            .rearrange("r (p t) -> p r t", t=LC))
                    nc.vector.tensor_copy(hwork[:], hwb[:])
                else:
                    nc.vector.tensor_copy(hwb[:], hwork[:])

                # ---- layer A (3-tile working range, 5-tile kv range) ----
                x5 = bigpool.tile([128, DT, P5], BF16, tag="x5")
                nc.vector.memset(x5[:, :, W5:P5], 0.0)
                layernorm(hwork, 0, hwb, x5, 0, W3, "a")
                layernorm(hwork, W3, hwb, x5, W3, 2 * LC, "b")

                # hA = h values of the a-range (f32), own tile exact
                hA = bigpool.tile([128, DT, W3], F32, tag="hA")
                nc.sync.dma_start(hA[:], hwork[:, :, bass.ds(arv * LC, W3)])
                # xq = x~ of the a-range
                xq = actpool.tile([128, DT, W3], BF16, tag="xq")
                nc.sync.dma_start(xq[:], x5[:, :, bass.ds(arv * LC, W3)])

                # K for 5(+1 pad) tiles, feature-major per head-pair
                K5 = []
                for g in range(DT):
                    ps = ppv.tile([128, D], F32, tag="ps_v", name="ps_k5")
                    for h3 in range(3):
                        for f in range(DT):
                            nc.tensor.matmul(
                                ps[:, h3 * 128:(h3 + 1) * 128],
                                w_qkv[:, f * 1536 + 512 + g * 128:
                                      f * 1536 + 512 + (g + 1) * 128],
                                x5[:, f, h3 * 128:(h3 + 1) * 128],
                                start=(f == 0), stop=(f == DT - 1))
                    K5_g = actpool.tile([128, P5], BF16, tag=f"K5{g}",
                                        name="K5_g")
                    nc.vector.tensor_scalar_add(
                        K5_g[:], ps[:, 0:P5],
                        pbc(lA, PB_QKVB + DT + g))
                    K5.append(K5_g)
                # V for 3 chunks of 128 tokens, token-major
                V5 = []
                for kc in range(3):
                    ps_v = ppv.tile([128, D], F32, tag="ps_v")
                    for f in range(DT):
                        nc.tensor.matmul(
                            ps_v[:], x5[:, f, kc * 128:(kc + 1) * 128],
                            w_qkv[:, f * 1536 + 1024:f * 1536 + 1536],
                            start=(f == 0), stop=False)
                    nc.tensor.matmul(ps_v[:], ones_row[:], vb[:],
                                     start=False, stop=True)
                    V5_kc = actpool.tile([128, D], BF16, tag=f"V5{kc}",
                                         name="V5_kc")
                    nc.vector.tensor_copy(V5_kc[:], ps_v[:])
                    V5.append(V5_kc)

                # q for the 3 a-tiles
                qT = []
                for g in range(DT):
                    ps = pp.tile([128, 2, W3], F32, tag="mm_out")
                    for f in range(DT):
                        nc.tensor.matmul(
                            ps[:, 0, :],
                            w_qkv[:, f * 1536 + g * 128:f * 1536 + (g + 1) * 128],
                            xq[:, f, :], start=(f == 0), stop=(f == DT - 1))
                    qT_g = actpool.tile([128, W3], BF16, tag=f"qT{g}",
                                        name="qT_g")
                    nc.vector.tensor_scalar_add(
                        qT_g[:], ps[:, 0, :], pbc(lA, PB_QKVB + g))
                    qT.append(qT_g)

                oT = [actpool.tile([128, W3], BF16, tag=f"oT{g}", name="oT_g")
                      for g in range(DT)]
                attention_A(K5, V5, qT, mA, oT)

                # proj + residual into hA
                for t in range(DT):
                    ps = pp.tile([128, 2, W3], F32, tag="mm_out")
                    for f in range(DT):
                        nc.tensor.matmul(
                            ps[:, 0, :],
                            w_proj[:, f * 512 + t * 128:f * 512 + (t + 1) * 128],
                            oT[f][:], start=(f == 0), stop=(f == DT - 1))
                    nc.vector.scalar_tensor_tensor(
                        hA[:, t, :], ps[:, 0, :], pbc(lA, PB_PROJB + t),
                        hA[:, t, :], op0=mybir.AluOpType.add,
                        op1=mybir.AluOpType.add)

                # LN2 + FFN on the 3 a-tiles
                hAb = bigpool.tile([128, DT, W3], BF16, tag="hAb")
                nc.vector.tensor_copy(hAb[:], hA[:])
                zA = bigpool.tile([128, DT, W3], BF16, tag="zA")
                layernorm(hA, 0, hAb, zA, 0, W3, "a")
                z1 = bigpool.tile([128, FT, W3], BF16, tag="z1")
                for ch in range(4):
                    wch = load_ff1_chunk(lA, ch)
                    for tl in range(4):
                        t = ch * 4 + tl
                        ps = pp.tile([128, 2, W3], F32, tag="mm_out",
                                     name="ps_ff1")
                        for f in range(DT):
                            nc.tensor.matmul(
                                ps[:, 0, :],
                                wch[:, tl * 512 + f * 128:
                                    tl * 512 + (f + 1) * 128],
                                zA[:, f, :], start=(f == 0),
                                stop=(f == DT - 1))
                        nc.scalar.activation(
                            z1[:, t, :], ps[:, 0, :],
                            mybir.ActivationFunctionType.Gelu,
                            bias=pb_all[:, lA, PB_FF1B + t:PB_FF1B + t + 1],
                            scale=1.0)
                for t in range(DT):
                    wch = load_ff2_chunk(lA, t)
                    ps = pp.tile([128, 2, W3], F32, tag="mm_out")
                    for g in range(FT):
                        nc.tensor.matmul(
                            ps[:, 0, :], wch[:, g * 128:(g + 1) * 128],
                            z1[:, g, :], start=(g == 0), stop=(g == FT - 1))
                    nc.vector.scalar_tensor_tensor(
                        hA[:, t, :], ps[:, 0, :], pbc(lA, PB_FF2B + t),
                        hA[:, t, :], op0=mybir.AluOpType.add,
                        op1=mybir.AluOpType.add)

                # ---- layer B (own tile; window = the 3 a-tiles) ----
                w_qkvB, w_projB, vbB = nxt
                if s + 1 < SEGS:
                    cur = load_layer(lA + 2)

                hAb2 = bigpool.tile([128, DT, W3], BF16, tag="hAb2")
                nc.vector.tensor_copy(hAb2[:], hA[:])
                x3 = bigpool.tile([128, DT, P3], BF16, tag="x3")
                nc.vector.memset(x3[:, :, W3:P3], 0.0)
                layernorm(hA, 0, hAb2, x3, 0, W3, "a")

                own_hA = actpool.tile([128, DT, LC], F32, tag="own_hA")
                nc.sync.dma_start(own_hA[:], hA[:, :, bass.ds(oiAv * LC, LC)])
                own_xq = actpool.tile([128, DT, LC], BF16, tag="own_xq")
                nc.sync.dma_start(own_xq[:], x3[:, :, bass.ds(oiAv * LC, LC)])

                K3 = []
                for g in range(DT):
                    ps = ppv.tile([128, D], F32, tag="ps_v", name="ps_k3")
                    for h2 in range(2):
                        for f in range(DT):
                            nc.tensor.matmul(
                                ps[:, h2 * 128:(h2 + 1) * 128],
                                w_qkvB[:, f * 1536 + 512 + g * 128:
                                       f * 1536 + 512 + (g + 1) * 128],
                                x3[:, f, h2 * 128:(h2 + 1) * 128],
                                start=(f == 0), stop=(f == DT - 1))
                    K3_g = actpool.tile([128, P3], BF16, tag=f"K3{g}",
                                        name="K3_g")
                    nc.vector.tensor_scalar_add(
                        K3_g[:], ps[:, 0:P3], pbc(lB, PB_QKVB + DT + g))
                    K3.append(K3_g)
                V3 = []
                for kc in range(2):
                    ps_v = ppv.tile([128, D], F32, tag="ps_v")
                    for f in range(DT):
                        nc.tensor.matmul(
                            ps_v[:], x3[:, f, kc * 128:(kc + 1) * 128],
                            w_qkvB[:, f * 1536 + 1024:f * 1536 + 1536],
                            start=(f == 0), stop=False)
                    nc.tensor.matmul(ps_v[:], ones_row[:], vbB[:],
                                     start=False, stop=True)
                    V3_kc = actpool.tile([128, D], BF16, tag=f"V3{kc}",
                                         name="V3_kc")
                    nc.vector.tensor_copy(V3_kc[:], ps_v[:])
                    V3.append(V3_kc)

                qTB = []
                for g in range(DT):
                    ps = pp.tile([128, 2, W3], F32, tag="mm_out")
                    for f in range(DT):
                        nc.tensor.matmul(
                            ps[:, 0, 0:LC],
                            w_qkvB[:, f * 1536 + g * 128:f * 1536 + (g + 1) * 128],
                            own_xq[:, f, :], start=(f == 0), stop=(f == DT - 1))
                    qTB_g = actpool.tile([128, LC], BF16, tag=f"qTB{g}",
                                         name="qTB_g")
                    nc.vector.tensor_scalar_add(
                        qTB_g[:], ps[:, 0, 0:LC], pbc(lB, PB_QKVB + g))
                    qTB.append(qTB_g)

                oTB = [actpool.tile([128, LC], BF16, tag=f"oTB{g}",
                                    name="oTB_g") for g in range(DT)]
                attention_B(K3, V3, qTB, mB, oTB)

                # proj + residual: hT_own = own_hA + proj(oTB) + b
                for t in range(DT):
                    ps = pp.tile([128, 2, W3], F32, tag="mm_out")
                    for f in range(DT):
                        nc.tensor.matmul(
                            ps[:, 0, 0:LC],
                            w_projB[:, f * 512 + t * 128:f * 512 + (t + 1) * 128],
                            oTB[f][:], start=(f == 0), stop=(f == DT - 1))
                    nc.vector.scalar_tensor_tensor(
                        hT_own[:, t, :], ps[:, 0, 0:LC], pbc(lB, PB_PROJB + t),
                        own_hA[:, t, :], op0=mybir.AluOpType.add,
                        op1=mybir.AluOpType.add)

                hTb = actpool.tile([128, DT, LC], BF16, tag="hTb")
                nc.vector.tensor_copy(hTb[:], hT_own[:])
                zB = bigpool.tile([128, DT, LC], BF16, tag="zB")
                layernorm(hT_own, 0, hTb, zB, 0, LC, "c")
                z1B = bigpool.tile([128, FT, LC], BF16, tag="z1B")
                for ch in range(4):
                    wch = load_ff1_chunk(lB, ch)
                    for tt in range(4):
                        t = ch * 4 + tt
                        ps = pp.tile([128, 2, W3], F32, tag="mm_out",
                                     name="ps_ff1b")
                        for f in range(DT):
                            nc.tensor.matmul(
                                ps[:, 0, 0:LC],
                                wch[:, tt * 512 + f * 128:
                                    tt * 512 + (f + 1) * 128],
                                zB[:, f, :], start=(f == 0),
                                stop=(f == DT - 1))
                        nc.scalar.activation(
                            z1B[:, t, :], ps[:, 0, 0:LC],
                            mybir.ActivationFunctionType.Gelu,
                            bias=pb_all[:, lB, PB_FF1B + t:PB_FF1B + t + 1],
                            scale=1.0)
                for t in range(DT):
                    wch = load_ff2_chunk(lB, t)
                    ps = pp.tile([128, 2, W3], F32, tag="mm_out")
                    for g in range(FT):
                        nc.tensor.matmul(
                            ps[:, 0, 0:LC], wch[:, g * 128:(g + 1) * 128],
                            z1B[:, g, :], start=(g == 0), stop=(g == FT - 1))
                    nc.vector.scalar_tensor_tensor(
                        hT_own[:, t, :], ps[:, 0, 0:LC], pbc(lB, PB_FF2B + t),
                        hT_own[:, t, :], op0=mybir.AluOpType.add,
                        op1=mybir.AluOpType.add)

                # ---- exchange h_{lB} (own tile) for the next segment ----
                if s + 1 < SEGS:
                    nxt = load_layer(lA + 3)
                    hob = actpool.tile([128, DT, LC], BF16, tag="hob")
                    nc.vector.tensor_copy(hob[:], hT_own[:])
                    ag_in = agdram.tile([D * LC], BF16, tag="ag_in")
                    ag_out_prev = agdram.tile([NC, D * LC], BF16, tag="ag_out",
                                              addr_space="Shared")
                    nc.sync.dma_start(
                        ag_in[:].rearrange("(f p t) -> p f t", p=128, t=LC),
                        hob[:])
                    nc.gpsimd.collective_compute(
                        "AllGather", mybir.AluOpType.bypass,
                        ins=[ag_in[:].opt()], outs=[ag_out_prev[:].opt()],
                        replica_groups=[list(range(NC))])

            # ---- output projection: y.T = tanh(out_w @ hT_own + out_b) ----
            hb = actpool.tile([128, DT, LC], BF16, tag="hb")
            nc.vector.tensor_copy(hb[:], hT_own[:])
            ps_y = pp.tile([128, 2, W3], F32, tag="mm_out", name="ps_y")
            for f in range(DT):
                nc.tensor.matmul(ps_y[0:PATCH, 0, 0:LC],
                                 wout_s[:, f * PATCH:(f + 1) * PATCH],
                                 hb[:, f, :], start=(f == 0), stop=(f == DT - 1))
            y_sb = actpool.tile([PATCH, LC], F32, tag="y_sb")
            nc.scalar.activation(y_sb[:], ps_y[0:PATCH, 0, 0:LC],
                                 mybir.ActivationFunctionType.Tanh,
                                 bias=outb_s[:, 0:1], scale=1.0)
            nc.sync.dma_start(yT[:], y_sb[:])

    nc.finalize()
    return nc


def _prep_inputs(inputs):
    """Host-side: pack full fp32 inputs into per-core in_maps."""
    I = {k: np.asarray(v, np.float32) for k, v in inputs.items()}

    scale = np.float32(DH ** -0.5)
    qkv_w = I["qkv_w"].copy()          # [LAYERS, 3D, D]
    qkv_b = I["qkv_b"].copy()          # [LAYERS, 3D]
    ff1_w = I["ff1_w"].copy()          # [LAYERS, DFF, D]
    ff1_b = I["ff1_b"].copy()          # [LAYERS, DFF]
    for l in range(LAYERS):
        qkv_b[l] += qkv_w[l] @ I["ln1_b"][l]
        qkv_w[l] *= I["ln1_g"][l][None, :]
        ff1_b[l] += ff1_w[l] @ I["ln2_b"][l]
        ff1_w[l] *= I["ln2_g"][l][None, :]
    qkv_w[:, :D] *= scale
    qkv_b[:, :D] *= scale

    def part_major(m):
        X = m.shape[0] // 128
        return np.ascontiguousarray(
            m.reshape(X, 128, m.shape[1]).transpose(1, 0, 2).reshape(128, -1))

    wblob = np.empty((LAYERS, 128, WCOLS), BF)
    pblob = np.zeros((LAYERS, 128, PCOLS), np.float32)
    for l in range(LAYERS):
        qkvT = np.ascontiguousarray(qkv_w[l].T)          # [D, 3D]
        projT = np.ascontiguousarray(I["proj_w"][l].T)   # [D, D]
        ff1T = np.ascontiguousarray(ff1_w[l].T)          # [D, DFF]
        ff2T = np.ascontiguousarray(I["ff2_w"][l].T)     # [DFF, D]
        wblob[l, :, OFF_QKV:OFF_PROJ] = part_major(qkvT).astype(BF)
        wblob[l, :, OFF_PROJ:OFF_FF1] = part_major(projT).astype(BF)
        # FF regions fo-major so they stream in per-fo chunks
        wblob[l, :, OFF_FF1:OFF_FF2] = (
            part_major(ff1T).reshape(128, 4, 16, 128)
            .transpose(0, 2, 1, 3).reshape(128, 8192).astype(BF))
        wblob[l, :, OFF_FF2:WCOLS] = (
            part_major(ff2T).reshape(128, 16, 4, 128)
            .transpose(0, 2, 1, 3).reshape(128, 8192).astype(BF))
        pblob[l, :, PB_QKVB:PB_QKVB + 12] = qkv_b[l].reshape(12, 128).T
        pblob[l, :, PB_PROJB:PB_PROJB + 4] = I["proj_b"][l].reshape(4, 128).T
        pblob[l, :, PB_FF1B:PB_FF1B + 16] = ff1_b[l].reshape(16, 128).T
        pblob[l, :, PB_FF2B:PB_FF2B + 4] = I["ff2_b"][l].reshape(4, 128).T
    vbias = np.ascontiguousarray(qkv_b[:, 2 * D:3 * D]).astype(BF)
    fbrow = ff1_b.astype(BF)

    # attention bias+mask table over global (key, query) pairs
    i = np.arange(L)
    ni = np.clip(i - K // 2, 0, L - K)
    k_idx = np.arange(L)[:, None]
    in_win = (k_idx >= ni[None, :]) & (k_idx < (ni + K)[None, :])
    rel = np.clip(k_idx - i[None, :] + (K - 1), 0, 2 * K - 2)
    rpb = I["rpb"]                                       # [LAYERS, H, 2K-1]
    B_full = np.where(in_win[None, None], rpb[:, :, rel],
                      np.float32(NEG)).astype(np.float32)  # [LAYERS,H,L,L]

    w_in_T = np.ascontiguousarray(I["in_w"].T).astype(BF)
    inb = np.ascontiguousarray(I["in_b"].reshape(DT, 128).T)
    out_wT = np.ascontiguousarray(I["out_w"].T)
    w_out = part_major(out_wT).astype(BF)
    outb = np.ascontiguousarray(I["out_b"].reshape(PATCH, 1))

    x_tok = I["x"].reshape(L, PATCH)                     # [L, PATCH]

    in_maps = []
    for c in range(NC):
        k0 = min(max(c - 2, 0), 3)
        a0 = min(max(c - 1, 0), 5)
        xT_c = np.ascontiguousarray(x_tok[c * LC:(c + 1) * LC].T).astype(BF)
        x5_c = np.ascontiguousarray(
            x_tok[k0 * LC:(k0 + 5) * LC].T).astype(BF)   # [PATCH, 320]

        # masks packed partition-major: mA[s, p, h, kc, j*64+qi]
        mA = np.full((SEGS, 128, H, 3, W3), NEG, np.float32)
        mB = np.full((SEGS, 128, H, 2, LC), NEG, np.float32)
        for s in range(SEGS):
            blkA = B_full[2 * s, :, k0 * LC:k0 * LC + W5, :]   # [H, 320, L]
            for j in range(3):
                t = a0 + j
                sl = blkA[:, :, t * LC:(t + 1) * LC]           # [H, 320, LC]
                for kc in range(3):
                    lo, hi = kc * 128, min((kc + 1) * 128, W5)
                    # [H, rows, LC] -> [rows, H, LC]
                    mA[s, 0:hi - lo, :, kc, j * LC:(j + 1) * LC] = (
                        sl[:, lo:hi].transpose(1, 0, 2))
            blkB = B_full[2 * s + 1, :, a0 * LC:a0 * LC + W3,
                          c * LC:(c + 1) * LC]                 # [H, 192, LC]
            for kc in range(2):
                lo, hi = kc * 128, min((kc + 1) * 128, W3)
                mB[s, 0:hi - lo, :, kc, :] = blkB[:, lo:hi].transpose(1, 0, 2)

        mA = np.exp(mA)
        mB = np.exp(mB)
        in_maps.append({
            "xT": xT_c,
            "x5in": x5_c,
            "w_in_T": w_in_T,
            "inb": inb,
            "wblob": wblob,
            "pblob": pblob,
            "vbias": vbias,
            "fbrow": fbrow,
            "maskA": mA.astype(BF),
            "maskB": mB.astype(BF),
            "w_out": w_out,
            "outb": outb,
        })
    return in_maps


def kernel(**inputs):
    if "nc" not in _BUILD_CACHE:
        _BUILD_CACHE["nc"] = _build()
    nc = _BUILD_CACHE["nc"]
    in_maps = _prep_inputs(inputs)
    res = run_bass_kernel_spmd(nc, in_maps, core_ids=list(range(NC)))
    y = np.empty((1, 1, L * PATCH), np.float32)
    for c in range(NC):
        yT_c = res.results[c]["yT"]                      # [PATCH, LC]
        y[0, 0, c * LC * PATCH:(c + 1) * LC * PATCH] = yT_c.T.reshape(-1)
    return y
